# revision 1
# baseline (speedup 1.0000x reference)
"""
Sparse (quantized) attention on 8 Trainium2 NeuronCores.

Strategy: head-parallel sharding. 16 (b,h) heads -> 2 heads per core, no
collectives. Per head the device computes, for each 128-query row-block
(causal: only the first rb+1 key tiles):

  scores = bf16-matmul(a_i*q_codes, ks_j*k_codes)        (exact-ish int codes)
         + bf16-matmul(K=6 hi/lo split of u_i*km_j+b_i*c_j)   [same PSUM]
  rowmax (DVE), e16 = exp(scores + ln16 - m)  (ACT, accum_out -> 16Z)
  f = relu((e16 + 8388607.5) - 8388608)  = floor(16*e)    (DVE TS + ACT relu)
  fT via PE transpose, PV = fT @ vd (bf16), out = PV * 1/(16Z)

This is exact in real arithmetic because for causal rows pmin=0, so
pd = floor(16*e)/(16*Z).  The single non-causal row (s=S-1) is computed
on the host. V dequant (v*vs+vm) is folded on the host into bf16 vd.
"""

import math
import os

import numpy as np
import ml_dtypes

S, B, H, D = 2048, 1, 16, 128
VG = 128
G = S // VG
P_LEVELS = 16.0
N_CORES = 8
HPC = H // N_CORES  # heads per core = 2
RB = 128            # row-block (query tile) size
NRB = S // RB       # 16 row-blocks
NKT = S // 128      # 16 key tiles

BF16 = ml_dtypes.bfloat16
MAGIC_A = 8388607.5   # 2^23 - 0.5 (exactly representable)
MAGIC_B = 8388608.0   # 2^23
DELTA = 1e-5          # nudges e_max just above 1 so floor(16*e_max)=16
LN16D = float(np.log(np.float64(16.0)) + DELTA)

_COMPILED = None  # cache (nc) across calls


def _build_graph():
    import concourse.bass as bass
    import concourse.bacc as bacc
    import concourse.tile as tile
    import concourse.mybir as mybir

    f32 = mybir.dt.float32
    bf16 = mybir.dt.bfloat16
    Alu = mybir.AluOpType
    Act = mybir.ActivationFunctionType

    nc = bacc.Bacc("TRN2", target_bir_lowering=False, debug=False,
                   num_devices=N_CORES)

    qT_d = nc.declare_dram_parameter("qT", [HPC, 128, S], bf16, isOutput=False)
    kTh_d = nc.declare_dram_parameter("kTh", [HPC, 128, S], bf16, isOutput=False)
    kTl_d = nc.declare_dram_parameter("kTl", [HPC, 128, S], bf16, isOutput=False)
    rho_d = nc.declare_dram_parameter("rho", [HPC, 128, NRB], f32, isOutput=False)
    nrho_d = nc.declare_dram_parameter("nrho", [HPC, 128, NRB], f32, isOutput=False)
    r2l_d = nc.declare_dram_parameter("r2l", [HPC, 6, S], bf16, isOutput=False)
    r2r_d = nc.declare_dram_parameter("r2r", [HPC, 6, S], bf16, isOutput=False)
    v_d = nc.declare_dram_parameter("vv", [HPC, 128, NKT, 128], bf16,
                                    isOutput=False)
    mask_d = nc.declare_dram_parameter("mask", [128, 128], f32, isOutput=False)
    negb_d = nc.declare_dram_parameter("negb", [128, 1], f32, isOutput=False)
    id_d = nc.declare_dram_parameter("ident", [128, 128], bf16, isOutput=False)
    out_d = nc.declare_dram_parameter("out", [HPC, NRB, 128, 128], f32,
                                      isOutput=True)

    CHUNK = int(os.environ.get("K_CHUNK", 8 * 128))
    PS_S = int(os.environ.get("K_PSS", 3))
    PS_T = int(os.environ.get("K_PST", 1))
    PS_V = int(os.environ.get("K_PSV", 1))
    WB = int(os.environ.get("K_WB", 3))

    with tile.TileContext(nc) as tc:
        with (
            tc.tile_pool(name="const", bufs=1) as constp,
            tc.tile_pool(name="heads", bufs=2) as headp,
            tc.tile_pool(name="work", bufs=WB) as workp,
            tc.tile_pool(name="stat", bufs=int(os.environ.get("K_SB", 6))) as statp,
            tc.tile_pool(name="ps_s", bufs=PS_S, space="PSUM") as ps_s,
            tc.tile_pool(name="ps_t", bufs=PS_T, space="PSUM") as ps_t,
            tc.tile_pool(name="ps_v", bufs=PS_V, space="PSUM") as ps_v,
        ):
            mask_sb = constp.tile([128, 128], f32, tag="mask")
            nc.sync.dma_start(mask_sb[:], mask_d[:])
            id_sb = constp.tile([128, 128], bf16, tag="ident")
            nc.sync.dma_start(id_sb[:], id_d[:])
            negb_sb = constp.tile([128, 1], f32, tag="negb")
            nc.sync.dma_start(negb_sb[:], negb_d[:])

            for h in range(HPC):
                qT_sb = headp.tile([128, S], bf16, tag="qT")
                nc.sync.dma_start(qT_sb[:], qT_d[h])
                kTh_sb = headp.tile([128, S], bf16, tag="kTh")
                nc.sync.dma_start(kTh_sb[:], kTh_d[h])
                kTl_sb = headp.tile([128, S], bf16, tag="kTl")
                nc.sync.dma_start(kTl_sb[:], kTl_d[h])
                rho_sb = headp.tile([128, NRB], f32, tag="rho")
                nc.sync.dma_start(rho_sb[:], rho_d[h])
                nrho_sb = headp.tile([128, NRB], f32, tag="nrho")
                nc.sync.dma_start(nrho_sb[:], nrho_d[h])
                r2l_sb = headp.tile([6, S], bf16, tag="r2l")
                nc.sync.dma_start(r2l_sb[:], r2l_d[h])
                r2r_sb = headp.tile([6, S], bf16, tag="r2r")
                nc.sync.dma_start(r2r_sb[:], r2r_d[h])
                v_sb = headp.tile([128, NKT, 128], bf16, tag="vv")
                nc.sync.dma_start(v_sb[:], v_d[h])

                for rb in range(NRB):
                    T = rb + 1          # key tiles for this row-block
                    NK = T * 128        # keys covered
                    q0 = rb * 128
                    nch = (NK + CHUNK - 1) // CHUNK

                    # scores in 1024-key psum chunks so the next row-block's
                    # matmuls can start while this one is still in softmax
                    chunks = []
                    mx = statp.tile([128, 4], f32, tag="mx")
                    for c in range(nch):
                        k0 = c * CHUNK
                        kn = min(NK, k0 + CHUNK) - k0
                        sc = ps_s.tile([128, CHUNK], f32, tag="sc")
                        chunks.append((sc, k0, kn))
                        for n0 in range(0, kn, 512):
                            n1 = min(kn, n0 + 512)
                            nc.tensor.matmul(sc[:, n0:n1],
                                             qT_sb[:, q0:q0 + 128],
                                             kTh_sb[:, k0 + n0:k0 + n1],
                                             start=True, stop=False)
                            nc.tensor.matmul(sc[:, n0:n1],
                                             qT_sb[:, q0:q0 + 128],
                                             kTl_sb[:, k0 + n0:k0 + n1],
                                             start=False, stop=False)
                            nc.tensor.matmul(sc[:, n0:n1],
                                             r2l_sb[:, q0:q0 + 128],
                                             r2r_sb[:, k0 + n0:k0 + n1],
                                             start=False, stop=True)
                        if k0 <= q0 < k0 + kn:
                            # causal mask on the diagonal tile
                            d0 = q0 - k0
                            nc.vector.tensor_add(sc[:, d0:d0 + 128],
                                                 sc[:, d0:d0 + 128],
                                                 mask_sb[:])
                        nc.vector.tensor_reduce(mx[:, c:c + 1], sc[:, :kn],
                                                axis=mybir.AxisListType.X,
                                                op=Alu.max)

                    m = statp.tile([128, 1], f32, tag="m")
                    nc.vector.tensor_reduce(m[:], mx[:, :nch],
                                            axis=mybir.AxisListType.X,
                                            op=Alu.max)
                    # nm = -rho*m + (ln16 + DELTA): e = 16*exp(s-m), nudged
                    # just high enough that floor(e_max) = 16 exactly
                    nm = statp.tile([128, 1], f32, tag="nm")
                    nc.vector.tensor_scalar(nm[:], m[:],
                                            nrho_sb[:, rb:rb + 1],
                                            DELTA,
                                            Alu.mult, Alu.add)

                    # e = 16*exp(...); zc = per-chunk sums; z = 16*Z
                    e = workp.tile([128, S], f32, tag="e")
                    zc = statp.tile([128, 4], f32, tag="zc")
                    for c, (sc, k0, kn) in enumerate(chunks):
                        nc.scalar.activation(e[:, k0:k0 + kn], sc[:, :kn],
                                             Act.Exp,
                                             bias=nm[:],
                                             scale=rho_sb[:, rb:rb + 1],
                                             accum_out=zc[:, c:c + 1])
                    z = statp.tile([128, 1], f32, tag="z")
                    nc.vector.tensor_reduce(z[:], zc[:, :nch],
                                            axis=mybir.AxisListType.X,
                                            op=Alu.add)
                    z16 = statp.tile([128, 1], f32, tag="z16")
                    nc.vector.tensor_scalar(z16[:], z[:], 16.0, None, Alu.mult)
                    r = statp.tile([128, 1], f32, tag="r")
                    nc.vector.reciprocal(r[:], z16[:])

                    # ft = max((16*e + (2^23-0.5)) - 2^23, 0) = floor(16e)
                    ft1 = workp.tile([128, S], f32, tag="ft1")
                    eng1 = nc.gpsimd if os.environ.get("K_TS1", "g") == "g" else nc.vector
                    eng1.tensor_scalar(ft1[:, :NK], e[:, :NK],
                                       16.0, MAGIC_A,
                                       Alu.mult, Alu.add)
                    ft = workp.tile([128, S], bf16, tag="ft")
                    eng = nc.gpsimd if os.environ.get("K_TS2", "g") == "g" else nc.vector
                    eng.tensor_scalar(ft[:, :NK], ft1[:, :NK],
                                      MAGIC_B, 0.0,
                                      Alu.subtract, Alu.max)

                    fT = workp.tile([128, S], bf16, tag="fT")
                    for bi, t0 in enumerate(range(0, T, 4)):
                        tn = min(4, T - t0)
                        ptr = ps_t.tile([128, 512], bf16, tag="tr")
                        for i in range(tn):
                            tt = t0 + i
                            nc.tensor.transpose(ptr[:, i * 128:(i + 1) * 128],
                                                ft[:, tt * 128:(tt + 1) * 128],
                                                id_sb[:])
                        dst = fT[:, t0 * 128:(t0 + tn) * 128]
                        cmode = os.environ.get("K_CP", "m")
                        use_v = (cmode == "v") or (cmode == "m" and bi % 2 == 0)
                        if use_v:
                            nc.vector.tensor_copy(dst, ptr[:, :tn * 128])
                        else:
                            nc.scalar.copy(dst, ptr[:, :tn * 128])

                    pv = ps_v.tile([128, 128], f32, tag="pv")
                    for t in range(T):
                        nc.tensor.matmul(pv[:],
                                         fT[:, t * 128:(t + 1) * 128],
                                         v_sb[:, t, :],
                                         start=(t == 0), stop=(t == T - 1))

                    o = workp.tile([128, 128], f32, tag="o")
                    if os.environ.get("K_OSC", "a") == "a":
                        nc.scalar.mul(o[:], pv[:], r[:])
                    else:
                        nc.vector.tensor_scalar(o[:], pv[:], r[:], None,
                                                Alu.mult)
                    nc.sync.dma_start(out_d[h, rb], o[:])

    nc.compile()
    return nc


def _host_prep(query, key, value, qmin, qscale, kmin, kscale, vmin, vscale):
    """Builds per-head device inputs. Returns dict of stacked arrays
    [H, ...] keyed like the dram params (per-head axis first)."""
    f32 = np.float32
    q = query[:, 0, :, :].astype(f32)     # [S, H, D]
    k = key[:, 0, :, :].astype(f32)
    v = value[:, 0, :, :].astype(f32)
    qs = qscale[:, 0, :].astype(f32)      # [S, H]
    qm = qmin[:, 0, :].astype(f32)
    ks = kscale[:, 0, :].astype(f32)
    km = kmin[:, 0, :].astype(f32)
    vs = vscale[:, 0, :, :].astype(f32)   # [G, H, D]
    vm = vmin[:, 0, :, :].astype(f32)

    rsd = f32(1.0 / math.sqrt(D))
    a = qs * rsd                          # [S, H]
    b = qm * rsd
    sq = q.sum(axis=2)                    # [S, H] exact ints in f32
    sk = k.sum(axis=2)
    u = a * sq + b * f32(D)               # [S, H]
    c = ks * sk                           # [S, H]

    # q side: exponent-exact folding. a = rho * 2^e, q'' = q * 2^e is
    # exactly representable in bf16 (codes < 128); rho in [1/sqrt2, sqrt2]
    # is applied per-row by the ACT scale operand.
    e_i = np.round(np.log2(a))
    two_e = np.exp2(e_i).astype(f32)      # [S, H]
    rho = (a / two_e).astype(f32)
    qT = np.ascontiguousarray((q * two_e[:, :, None]).transpose(1, 2, 0)
                              ).astype(BF16)        # [H, D, S]

    # k side: hi/lo split of ks*k (sum of two bf16 matmuls is exact)
    kp = (k * ks[:, :, None]).astype(f32)
    kph = kp.astype(BF16)
    kpl = (kp - kph.astype(f32)).astype(BF16)
    kTh = np.ascontiguousarray(kph.transpose(1, 2, 0))  # [H, D, S]
    kTl = np.ascontiguousarray(kpl.transpose(1, 2, 0))

    # rho laid out [H, 128(partition=q within block), NRB]
    rho_r = np.ascontiguousarray(
        rho.T.reshape(H, NRB, 128).transpose(0, 2, 1)).astype(f32)
    nrho_r = np.ascontiguousarray(-rho_r)

    def hilo(x):
        xh = x.astype(BF16).astype(f32)
        xl = (x - xh).astype(BF16).astype(f32)
        return xh, xl

    up = (u / rho).astype(f32)
    bp = (b / rho).astype(f32)
    uh, ul = hilo(up)
    bh, bl = hilo(bp)
    kmh, kml = hilo(km)
    ch, cl = hilo(c)
    # (u'/rho)*km + (b'/rho)*c ~= uh*kmh + uh*kml + ul*kmh + bh*ch + bh*cl + bl*ch
    r2l = np.stack([uh, uh, ul, bh, bh, bl], axis=0)     # [6, S, H]
    r2r = np.stack([kmh, kml, kmh, ch, cl, ch], axis=0)  # [6, S, H]
    r2l = np.ascontiguousarray(r2l.transpose(2, 0, 1)).astype(BF16)  # [H,6,S]
    r2r = np.ascontiguousarray(r2r.transpose(2, 0, 1)).astype(BF16)

    # dequantized V: [S, H, D] -> per head [128, NKT, 128]
    vs_full = np.repeat(vs, VG, axis=0)   # [S, H, D]
    vm_full = np.repeat(vm, VG, axis=0)
    vd = v * vs_full + vm_full            # f32 [S, H, D]
    vdt = vd.transpose(1, 0, 2).reshape(H, NKT, 128, D)  # [H, kt, t, D]
    vdt = np.ascontiguousarray(vdt.transpose(0, 2, 1, 3)).astype(BF16)

    mask = np.triu(np.full((128, 128), -1e30, dtype=f32), k=1)
    ident = np.eye(128, dtype=np.float32).astype(BF16)

    return dict(qT=qT, kTh=kTh, kTl=kTl, rho=rho_r, nrho=nrho_r,
                r2l=r2l, r2r=r2r, vv=vdt, mask=mask,
                ident=ident, vd_f32=vd,
                negb=np.full((128, 1), -MAGIC_B, dtype=f32))


def _host_last_row(query, key, qmin, qscale, kmin, kscale, vd_f32):
    """Exact reference math (numpy f32) for the single non-causal row
    s = S-1, all heads. Returns [H, D]."""
    f32 = np.float32
    i = S - 1
    out = np.zeros((H, D), dtype=f32)
    for h in range(H):
        qd = query[i, 0, h, :].astype(f32) * f32(qscale[i, 0, h]) + f32(qmin[i, 0, h])
        kd = key[:, 0, h, :].astype(f32) * kscale[:, 0, h].astype(f32)[:, None] \
            + kmin[:, 0, h].astype(f32)[:, None]
        s = (kd @ qd).astype(f32) * f32(1.0 / math.sqrt(D))   # [S]
        e = np.exp(s - s.max(), dtype=f32)
        p = (e / e.sum(dtype=f32)).astype(f32)
        pmax, pmin_ = p.max(), p.min()
        pscale = (pmax - pmin_) / f32(P_LEVELS)
        safe = pscale if pscale > 0 else f32(1.0)
        pq = np.floor((p - pmin_) / safe).astype(f32)
        pd = pq * pscale + pmin_
        out[h] = pd @ vd_f32[:, h, :]
    return out


def _reference_numpy(query, key, value, qmin, qscale, kmin, kscale,
                     vmin, vscale, causal):
    """Full-precision numpy fallback (used only if causal != 1)."""
    f32 = np.float32
    q = query[:, 0, :, :].astype(f32)
    k = key[:, 0, :, :].astype(f32)
    v = value[:, 0, :, :].astype(f32)
    out = np.zeros((S, B, H * D), dtype=f32)
    vs_full = np.repeat(vscale[:, 0, :, :].astype(f32), VG, axis=0)
    vm_full = np.repeat(vmin[:, 0, :, :].astype(f32), VG, axis=0)
    for h in range(H):
        qd = q[:, h, :] * qscale[:, 0, h].astype(f32)[:, None] + qmin[:, 0, h].astype(f32)[:, None]
        kd = k[:, h, :] * kscale[:, 0, h].astype(f32)[:, None] + kmin[:, 0, h].astype(f32)[:, None]
        s = (qd @ kd.T) * f32(1.0 / math.sqrt(D))
        if causal:
            s = np.where(np.tril(np.ones((S, S), dtype=bool)), s, f32(-1e30))
        e = np.exp(s - s.max(axis=1, keepdims=True), dtype=f32)
        p = e / e.sum(axis=1, keepdims=True, dtype=f32)
        pmax = p.max(axis=1, keepdims=True)
        pmin_ = p.min(axis=1, keepdims=True)
        pscale = (pmax - pmin_) / f32(P_LEVELS)
        safe = np.where(pscale > 0, pscale, f32(1.0))
        pd = np.floor((p - pmin_) / safe) * pscale + pmin_
        vd = v[:, h, :] * vs_full[:, h, :] + vm_full[:, h, :]
        out[:, 0, h * D:(h + 1) * D] = pd.astype(f32) @ vd
    return out


def kernel(query, key, value, qmin, qscale, kmin, kscale, vmin, vscale,
           causal):
    global _COMPILED
    causal_i = int(np.asarray(causal))
    if causal_i != 1:
        return _reference_numpy(query, key, value, qmin, qscale, kmin,
                                kscale, vmin, vscale, causal_i)

    prep = _host_prep(query, key, value, qmin, qscale, kmin, kscale,
                      vmin, vscale)

    if _COMPILED is None:
        _COMPILED = _build_graph()
    nc = _COMPILED

    in_maps = []
    for core in range(N_CORES):
        hs = slice(core * HPC, (core + 1) * HPC)
        in_maps.append({
            "qT": np.ascontiguousarray(prep["qT"][hs]),
            "kTh": np.ascontiguousarray(prep["kTh"][hs]),
            "kTl": np.ascontiguousarray(prep["kTl"][hs]),
            "rho": np.ascontiguousarray(prep["rho"][hs]),
            "nrho": np.ascontiguousarray(prep["nrho"][hs]),
            "r2l": np.ascontiguousarray(prep["r2l"][hs]),
            "r2r": np.ascontiguousarray(prep["r2r"][hs]),
            "vv": np.ascontiguousarray(prep["vv"][hs]),
            "mask": prep["mask"],
            "ident": prep["ident"],
            "negb": prep["negb"],
        })

    from concourse.bass_utils import run_bass_kernel_spmd
    trace = bool(int(os.environ.get("KERNEL_TRACE", "0")))
    res = run_bass_kernel_spmd(nc, in_maps, core_ids=list(range(N_CORES)),
                               trace=trace)
    if res.exec_time_ns is not None:
        kernel.last_exec_ns = res.exec_time_ns
        print(f"HW exec time: {res.exec_time_ns} ns")

    out = np.zeros((S, B, H * D), dtype=np.float32)
    for core in range(N_CORES):
        o = np.asarray(res.results[core]["out"], dtype=np.float32)
        # o: [HPC, NRB, 128, 128]
        for j in range(HPC):
            h = core * HPC + j
            out[:, 0, h * D:(h + 1) * D] = o[j].reshape(S, D)

    # fix up the one non-causal row (pmin != 0 there)
    last = _host_last_row(query, key, qmin, qscale, kmin, kscale,
                          prep["vd_f32"])
    for h in range(H):
        out[S - 1, 0, h * D:(h + 1) * D] = last[h]
    return out


kernel.last_exec_ns = None



# revision 8
# speedup vs baseline: 1.0729x; 1.0729x over previous
"""
Sparse (quantized) attention on 8 Trainium2 NeuronCores.

Strategy: head-parallel sharding. 16 (b,h) heads -> 2 heads per core, no
collectives. Per head the device computes, for each 128-query row-block
(causal: only the first rb+1 key tiles):

  scores*2^18 in PSUM via
    P1: bf16-matmul(q*2^e, 2^18*bf16(ks*k))            (exact products)
    P2: fp8e4 DoubleRow matmul of the (q hi,lo) pair against the
        duplicated fp8 k-lo residual (2^18 split 2^8 * 2^10)
    P3: rank-6 bf16 correction (u/rho)*km + (b/rho)*c  (hi/lo split)
  row max m (DVE per-chunk reduce), nm = -rho'*m + ln16 + delta
  e16 = exp(rho'*sc + nm)  in [0, 16.003], accum_out -> z = 16*Z
  t = bf16(e16 + 127.5)    (Pool; bf16 round-to-nearest == floor+128)
  fT via PE transpose of t, eviction = relu(t^ - 128) (ACT/DVE split)
  PV = fT @ vd (bf16), out = PV * (1/z)

Exact in real arithmetic because for causal rows pmin=0, so
pd = floor(16*e)/(16*Z).  The single non-causal row (s=S-1) is computed
on the host. V dequant (v*vs+vm) is folded on the host into bf16 vd.
"""

import math
import os

import numpy as np
import ml_dtypes

S, B, H, D = 2048, 1, 16, 128
VG = 128
G = S // VG
P_LEVELS = 16.0
N_CORES = 8
HPC = H // N_CORES  # heads per core = 2
RB = 128            # row-block (query tile) size
NRB = S // RB       # 16 row-blocks
NKT = S // 128      # 16 key tiles

BF16 = ml_dtypes.bfloat16
FP8E4 = ml_dtypes.float8_e4m3
DELTA = 2e-4
LN16D = float(np.log(np.float64(16.0)) + DELTA)
GS = 18             # global log2 scale on the scores PSUM
QS8 = 8             # q-pair fp8 pre-scale (2^QS8)
KS8 = GS - QS8      # k-lo fp8 pre-scale

_COMPILED = None


def _build_graph():
    import concourse.bass as bass
    import concourse.bacc as bacc
    import concourse.tile as tile
    import concourse.mybir as mybir

    f32 = mybir.dt.float32
    bf16 = mybir.dt.bfloat16
    fp8e4 = mybir.dt.float8e4
    Alu = mybir.AluOpType
    Act = mybir.ActivationFunctionType

    nc = bacc.Bacc("TRN2", target_bir_lowering=False, debug=False,
                   num_devices=N_CORES)

    qT_d = nc.declare_dram_parameter("qT", [HPC, 128, S], bf16, isOutput=False)
    kTh_d = nc.declare_dram_parameter("kTh", [HPC, 128, S], bf16, isOutput=False)
    qp8_d = nc.declare_dram_parameter("qp8", [HPC, 128, 2, S], fp8e4,
                                      isOutput=False)
    kl8_d = nc.declare_dram_parameter("kl8", [HPC, 128, 2, S], fp8e4,
                                      isOutput=False)
    rho_d = nc.declare_dram_parameter("rho", [HPC, 128, NRB], f32, isOutput=False)
    nrho_d = nc.declare_dram_parameter("nrho", [HPC, 128, NRB], f32, isOutput=False)
    r2l_d = nc.declare_dram_parameter("r2l", [HPC, 6, S], bf16, isOutput=False)
    r2r_d = nc.declare_dram_parameter("r2r", [HPC, 6, S], bf16, isOutput=False)
    v_d = nc.declare_dram_parameter("vv", [HPC, 128, NKT, 128], bf16,
                                    isOutput=False)
    mask_d = nc.declare_dram_parameter("mask", [128, 128], f32, isOutput=False)
    n128_d = nc.declare_dram_parameter("neg128", [128, 1], f32, isOutput=False)
    id_d = nc.declare_dram_parameter("ident", [128, 128], bf16, isOutput=False)
    out_d = nc.declare_dram_parameter("out", [HPC, NRB, 128, 128], f32,
                                      isOutput=True)

    CHUNK = int(os.environ.get("K_CHUNK", 1024))
    PS_S = int(os.environ.get("K_PSS", 3))
    PS_T = int(os.environ.get("K_PST", 1))
    PS_V = int(os.environ.get("K_PSV", 1))
    WB = int(os.environ.get("K_WB", 3))
    TG = int(os.environ.get("K_TG", 8))          # tiles per transpose group
    EV_MOD = int(os.environ.get("K_EVM", 3))     # eviction: gi%EV_MOD==EV_ACT -> ACT
    EV_ACT = int(os.environ.get("K_EVA", 2))
    MG_ENG = os.environ.get("K_MG", "g")         # magic-add engine: g=Pool
    USE_FP8_P2 = int(os.environ.get("K_FP8P2", 1))

    with tile.TileContext(nc) as tc:
        with (
            tc.tile_pool(name="const", bufs=1) as constp,
            tc.tile_pool(name="heads", bufs=2) as headp,
            tc.tile_pool(name="work", bufs=WB) as workp,
            tc.tile_pool(name="stat", bufs=int(os.environ.get("K_SB", 6))) as statp,
            tc.tile_pool(name="ps_s", bufs=PS_S, space="PSUM") as ps_s,
            tc.tile_pool(name="ps_t", bufs=PS_T, space="PSUM") as ps_t,
            tc.tile_pool(name="ps_v", bufs=PS_V, space="PSUM") as ps_v,
        ):
            mask_sb = constp.tile([128, 128], f32, tag="mask")
            nc.sync.dma_start(mask_sb[:], mask_d[:])
            n128_sb = constp.tile([128, 1], f32, tag="neg128")
            nc.sync.dma_start(n128_sb[:], n128_d[:])
            id_sb = constp.tile([128, 128], bf16, tag="ident")
            nc.sync.dma_start(id_sb[:], id_d[:])

            gi = 0  # global transpose-group counter (eviction engine split)
            for h in range(HPC):
                qT_sb = headp.tile([128, S], bf16, tag="qT")
                nc.sync.dma_start(qT_sb[:], qT_d[h])
                kTh_sb = headp.tile([128, S], bf16, tag="kTh")
                nc.sync.dma_start(kTh_sb[:], kTh_d[h])
                if USE_FP8_P2:
                    qp8_sb = headp.tile([128, 2, S], fp8e4, tag="qp8")
                    nc.sync.dma_start(qp8_sb[:], qp8_d[h])
                    kl8_sb = headp.tile([128, 2, S], fp8e4, tag="kl8")
                    nc.sync.dma_start(kl8_sb[:], kl8_d[h])
                else:
                    kTl_sb = headp.tile([128, S], bf16, tag="kTl")
                    nc.sync.dma_start(kTl_sb[:], qp8_d[h])  # unused path
                rho_sb = headp.tile([128, NRB], f32, tag="rho")
                nc.sync.dma_start(rho_sb[:], rho_d[h])
                nrho_sb = headp.tile([128, NRB], f32, tag="nrho")
                nc.sync.dma_start(nrho_sb[:], nrho_d[h])
                r2l_sb = headp.tile([6, S], bf16, tag="r2l")
                nc.sync.dma_start(r2l_sb[:], r2l_d[h])
                r2r_sb = headp.tile([6, S], bf16, tag="r2r")
                nc.sync.dma_start(r2r_sb[:], r2r_d[h])
                v_sb = headp.tile([128, NKT, 128], bf16, tag="vv")
                nc.sync.dma_start(v_sb[:], v_d[h])

                for rb in range(NRB):
                    T = rb + 1
                    NK = T * 128
                    q0 = rb * 128
                    nch = (NK + CHUNK - 1) // CHUNK

                    chunks = []
                    mx = statp.tile([128, 4], f32, tag="mx")
                    for c in range(nch):
                        k0 = c * CHUNK
                        kn = min(NK, k0 + CHUNK) - k0
                        sc = ps_s.tile([128, CHUNK], f32, tag="sc")
                        chunks.append((sc, k0, kn))
                        for n0 in range(0, kn, 512):
                            n1 = min(kn, n0 + 512)
                            nc.tensor.matmul(sc[:, n0:n1],
                                             qT_sb[:, q0:q0 + 128],
                                             kTh_sb[:, k0 + n0:k0 + n1],
                                             start=True, stop=False)
                            if USE_FP8_P2:
                                nc.tensor.matmul(
                                    sc[:, n0:n1],
                                    qp8_sb[:, :, q0:q0 + 128],
                                    kl8_sb[:, :, k0 + n0:k0 + n1],
                                    start=False, stop=False,
                                    perf_mode=mybir.MatmulPerfMode.DoubleRow)
                            else:
                                nc.tensor.matmul(sc[:, n0:n1],
                                                 qT_sb[:, q0:q0 + 128],
                                                 kTl_sb[:, k0 + n0:k0 + n1],
                                                 start=False, stop=False)
                            nc.tensor.matmul(sc[:, n0:n1],
                                             r2l_sb[:, q0:q0 + 128],
                                             r2r_sb[:, k0 + n0:k0 + n1],
                                             start=False, stop=True)
                        if k0 <= q0 < k0 + kn:
                            d0 = q0 - k0
                            nc.vector.tensor_add(sc[:, d0:d0 + 128],
                                                 sc[:, d0:d0 + 128],
                                                 mask_sb[:])
                        nc.vector.tensor_reduce(mx[:, c:c + 1], sc[:, :kn],
                                                axis=mybir.AxisListType.X,
                                                op=Alu.max)

                    m = statp.tile([128, 1], f32, tag="m")
                    nc.vector.tensor_reduce(m[:], mx[:, :nch],
                                            axis=mybir.AxisListType.X,
                                            op=Alu.max)
                    nm = statp.tile([128, 1], f32, tag="nm")
                    nc.vector.tensor_scalar(nm[:], m[:],
                                            nrho_sb[:, rb:rb + 1],
                                            LN16D,
                                            Alu.mult, Alu.add)

                    # e16 = 16*exp(s-m+delta); zc = per-chunk sums
                    e = workp.tile([128, S], f32, tag="e")
                    zc = statp.tile([128, 4], f32, tag="zc")
                    for c, (sc, k0, kn) in enumerate(chunks):
                        nc.scalar.activation(e[:, k0:k0 + kn], sc[:, :kn],
                                             Act.Exp,
                                             bias=nm[:],
                                             scale=rho_sb[:, rb:rb + 1],
                                             accum_out=zc[:, c:c + 1])
                    z = statp.tile([128, 1], f32, tag="z")
                    nc.vector.tensor_reduce(z[:], zc[:, :nch],
                                            axis=mybir.AxisListType.X,
                                            op=Alu.add)
                    r = statp.tile([128, 1], f32, tag="r")
                    nc.vector.reciprocal(r[:], z[:])

                    # t = bf16(e16 + 127.5): bf16 RN in [128,256) == floor+128
                    t = workp.tile([128, S], bf16, tag="t")
                    for c, (sc, k0, kn) in enumerate(chunks):
                        eng = nc.gpsimd if MG_ENG == "g" else nc.vector
                        eng.tensor_scalar(t[:, k0:k0 + kn], e[:, k0:k0 + kn],
                                          127.5, None, Alu.add)

                    # transpose 128x128 tiles; evict with relu(t^ - 128)
                    fT = workp.tile([128, S], bf16, tag="fT")
                    for t0 in range(0, T, TG):
                        tn = min(TG, T - t0)
                        ptr = ps_t.tile([128, TG * 128], bf16, tag="tr")
                        for i in range(tn):
                            tt = t0 + i
                            nc.tensor.transpose(ptr[:, i * 128:(i + 1) * 128],
                                                t[:, tt * 128:(tt + 1) * 128],
                                                id_sb[:])
                        dst = fT[:, t0 * 128:(t0 + tn) * 128]
                        if gi % EV_MOD == EV_ACT:
                            nc.scalar.activation(dst, ptr[:, :tn * 128],
                                                 Act.Relu,
                                                 bias=n128_sb[:], scale=1.0)
                        else:
                            nc.vector.tensor_scalar(dst, ptr[:, :tn * 128],
                                                    128.0, 0.0,
                                                    Alu.subtract, Alu.max)
                        gi += 1

                    pv = ps_v.tile([128, 128], f32, tag="pv")
                    for tt in range(T):
                        nc.tensor.matmul(pv[:],
                                         fT[:, tt * 128:(tt + 1) * 128],
                                         v_sb[:, tt, :],
                                         start=(tt == 0), stop=(tt == T - 1))

                    o = workp.tile([128, 128], f32, tag="o")
                    if os.environ.get("K_OSC", "a") == "a":
                        nc.scalar.mul(o[:], pv[:], r[:])
                    else:
                        nc.vector.tensor_scalar(o[:], pv[:], r[:], None,
                                                Alu.mult)
                    nc.sync.dma_start(out_d[h, rb], o[:])

    nc.compile()
    return nc


def _host_prep(query, key, value, qmin, qscale, kmin, kscale, vmin, vscale):
    """Builds per-head device inputs, stacked [H, ...]."""
    f32 = np.float32
    q = query[:, 0, :, :].astype(f32)     # [S, H, D]
    k = key[:, 0, :, :].astype(f32)
    v = value[:, 0, :, :].astype(f32)
    qs = qscale[:, 0, :].astype(f32)      # [S, H]
    qm = qmin[:, 0, :].astype(f32)
    ks = kscale[:, 0, :].astype(f32)
    km = kmin[:, 0, :].astype(f32)
    vs = vscale[:, 0, :, :].astype(f32)   # [G, H, D]
    vm = vmin[:, 0, :, :].astype(f32)

    rsd = f32(1.0 / math.sqrt(D))
    a = qs * rsd
    b = qm * rsd
    sq = q.sum(axis=2)
    sk = k.sum(axis=2)
    u = a * sq + b * f32(D)
    c = ks * sk

    # q side: a = rho * 2^e; q2e = q * 2^e exact in bf16.
    e_i = np.round(np.log2(a))
    two_e = np.exp2(e_i).astype(f32)
    rho = (a / two_e).astype(f32)
    q2e = q * two_e[:, :, None]                         # [S, H, D] exact
    qT = np.ascontiguousarray(q2e.transpose(1, 2, 0)).astype(BF16)  # [H,D,S]

    # scores PSUM is globally scaled by 2^GS; rho' = rho * 2^-GS
    rho_s = (rho * f32(2.0 ** -GS)).astype(f32)

    # k hi: bf16(ks*k) * 2^GS (exact exponent shift after rounding)
    kp = (k * ks[:, :, None]).astype(f32)
    kph = kp.astype(BF16).astype(f32)
    kTh = np.ascontiguousarray((kph * f32(2.0 ** GS)).transpose(1, 2, 0)
                               ).astype(BF16)           # [H, D, S]

    # k lo residual as fp8 pair path: q-pair (hi/lo nibbles of q2e * 2^QS8)
    q2e_s = (q2e * f32(2.0 ** QS8)).astype(f32)
    q1 = q2e_s.astype(FP8E4).astype(f32)
    q2 = (q2e_s - q1).astype(FP8E4)
    qp8 = np.stack([q1.astype(FP8E4), q2], axis=2)      # [S, H, 2, D]
    qp8 = np.ascontiguousarray(qp8.transpose(1, 3, 2, 0))  # [H, D, 2, S]

    kpl = ((kp - kph) * f32(2.0 ** KS8)).astype(FP8E4)  # [S, H, D]
    kl8 = np.stack([kpl, kpl], axis=2)                  # [S, H, 2, D]
    kl8 = np.ascontiguousarray(kl8.transpose(1, 3, 2, 0))  # [H, D, 2, S]

    rho_r = np.ascontiguousarray(
        rho_s.T.reshape(H, NRB, 128).transpose(0, 2, 1)).astype(f32)
    nrho_r = np.ascontiguousarray(-rho_r)

    def hilo(x):
        xh = x.astype(BF16).astype(f32)
        xl = (x - xh).astype(BF16).astype(f32)
        return xh, xl

    # rank-6 correction, scaled 2^GS split as 2^9 per side
    s9 = f32(2.0 ** 9)
    up = (u / rho * s9).astype(f32)
    bp = (b / rho * s9).astype(f32)
    kms = (km * s9).astype(f32)
    cs = (c * s9).astype(f32)
    uh, ul = hilo(up)
    bh, bl = hilo(bp)
    kmh, kml = hilo(kms)
    ch, cl = hilo(cs)
    r2l = np.stack([uh, uh, ul, bh, bh, bl], axis=0)     # [6, S, H]
    r2r = np.stack([kmh, kml, kmh, ch, cl, ch], axis=0)
    r2l = np.ascontiguousarray(r2l.transpose(2, 0, 1)).astype(BF16)  # [H,6,S]
    r2r = np.ascontiguousarray(r2r.transpose(2, 0, 1)).astype(BF16)

    vs_full = np.repeat(vs, VG, axis=0)
    vm_full = np.repeat(vm, VG, axis=0)
    vd = v * vs_full + vm_full            # f32 [S, H, D]
    vdt = vd.transpose(1, 0, 2).reshape(H, NKT, 128, D)
    vdt = np.ascontiguousarray(vdt.transpose(0, 2, 1, 3)).astype(BF16)

    mask = np.triu(np.full((128, 128), -1e30, dtype=f32), k=1)
    ident = np.eye(128, dtype=np.float32).astype(BF16)

    return dict(qT=qT, kTh=kTh, qp8=qp8, kl8=kl8, rho=rho_r, nrho=nrho_r,
                r2l=r2l, r2r=r2r, vv=vdt, mask=mask,
                neg128=np.full((128, 1), -128.0, dtype=f32),
                ident=ident, vd_f32=vd)


def _host_last_row(query, key, qmin, qscale, kmin, kscale, vd_f32):
    """Exact reference math (numpy f32) for the single non-causal row."""
    f32 = np.float32
    i = S - 1
    out = np.zeros((H, D), dtype=f32)
    for h in range(H):
        qd = query[i, 0, h, :].astype(f32) * f32(qscale[i, 0, h]) + f32(qmin[i, 0, h])
        kd = key[:, 0, h, :].astype(f32) * kscale[:, 0, h].astype(f32)[:, None] \
            + kmin[:, 0, h].astype(f32)[:, None]
        s = (kd @ qd).astype(f32) * f32(1.0 / math.sqrt(D))
        e = np.exp(s - s.max(), dtype=f32)
        p = (e / e.sum(dtype=f32)).astype(f32)
        pmax, pmin_ = p.max(), p.min()
        pscale = (pmax - pmin_) / f32(P_LEVELS)
        safe = pscale if pscale > 0 else f32(1.0)
        pq = np.floor((p - pmin_) / safe).astype(f32)
        pd = pq * pscale + pmin_
        out[h] = pd @ vd_f32[:, h, :]
    return out


def _reference_numpy(query, key, value, qmin, qscale, kmin, kscale,
                     vmin, vscale, causal):
    f32 = np.float32
    q = query[:, 0, :, :].astype(f32)
    k = key[:, 0, :, :].astype(f32)
    v = value[:, 0, :, :].astype(f32)
    out = np.zeros((S, B, H * D), dtype=f32)
    vs_full = np.repeat(vscale[:, 0, :, :].astype(f32), VG, axis=0)
    vm_full = np.repeat(vmin[:, 0, :, :].astype(f32), VG, axis=0)
    for h in range(H):
        qd = q[:, h, :] * qscale[:, 0, h].astype(f32)[:, None] + qmin[:, 0, h].astype(f32)[:, None]
        kd = k[:, h, :] * kscale[:, 0, h].astype(f32)[:, None] + kmin[:, 0, h].astype(f32)[:, None]
        s = (qd @ kd.T) * f32(1.0 / math.sqrt(D))
        if causal:
            s = np.where(np.tril(np.ones((S, S), dtype=bool)), s, f32(-1e30))
        e = np.exp(s - s.max(axis=1, keepdims=True), dtype=f32)
        p = e / e.sum(axis=1, keepdims=True, dtype=f32)
        pmax = p.max(axis=1, keepdims=True)
        pmin_ = p.min(axis=1, keepdims=True)
        pscale = (pmax - pmin_) / f32(P_LEVELS)
        safe = np.where(pscale > 0, pscale, f32(1.0))
        pd = np.floor((p - pmin_) / safe) * pscale + pmin_
        vd = v[:, h, :] * vs_full[:, h, :] + vm_full[:, h, :]
        out[:, 0, h * D:(h + 1) * D] = pd.astype(f32) @ vd
    return out


def kernel(query, key, value, qmin, qscale, kmin, kscale, vmin, vscale,
           causal):
    global _COMPILED
    causal_i = int(np.asarray(causal))
    if causal_i != 1:
        return _reference_numpy(query, key, value, qmin, qscale, kmin,
                                kscale, vmin, vscale, causal_i)

    prep = _host_prep(query, key, value, qmin, qscale, kmin, kscale,
                      vmin, vscale)

    if _COMPILED is None:
        _COMPILED = _build_graph()
    nc = _COMPILED

    in_maps = []
    for core in range(N_CORES):
        hs = slice(core * HPC, (core + 1) * HPC)
        in_maps.append({
            "qT": np.ascontiguousarray(prep["qT"][hs]),
            "kTh": np.ascontiguousarray(prep["kTh"][hs]),
            "qp8": np.ascontiguousarray(prep["qp8"][hs]),
            "kl8": np.ascontiguousarray(prep["kl8"][hs]),
            "rho": np.ascontiguousarray(prep["rho"][hs]),
            "nrho": np.ascontiguousarray(prep["nrho"][hs]),
            "r2l": np.ascontiguousarray(prep["r2l"][hs]),
            "r2r": np.ascontiguousarray(prep["r2r"][hs]),
            "vv": np.ascontiguousarray(prep["vv"][hs]),
            "mask": prep["mask"],
            "neg128": prep["neg128"],
            "ident": prep["ident"],
        })

    from concourse.bass_utils import run_bass_kernel_spmd
    trace = bool(int(os.environ.get("KERNEL_TRACE", "0")))
    res = run_bass_kernel_spmd(nc, in_maps, core_ids=list(range(N_CORES)),
                               trace=trace)
    if res.exec_time_ns is not None:
        kernel.last_exec_ns = res.exec_time_ns
        print(f"HW exec time: {res.exec_time_ns} ns")

    out = np.zeros((S, B, H * D), dtype=np.float32)
    for core in range(N_CORES):
        o = np.asarray(res.results[core]["out"], dtype=np.float32)
        for j in range(HPC):
            h = core * HPC + j
            out[:, 0, h * D:(h + 1) * D] = o[j].reshape(S, D)

    last = _host_last_row(query, key, qmin, qscale, kmin, kscale,
                          prep["vd_f32"])
    for h in range(H):
        out[S - 1, 0, h * D:(h + 1) * D] = last[h]
    return out


kernel.last_exec_ns = None


# revision 10
# speedup vs baseline: 1.3014x; 1.2130x over previous
"""
Sparse (quantized) attention on 8 Trainium2 NeuronCores.

Strategy: head-parallel sharding. 16 (b,h) heads -> 2 heads per core, no
collectives. Per head the device computes, for each 128-query row-block
(causal: only the first rb+1 key tiles):

  scores*2^18 in PSUM via
    P1: bf16-matmul(q*2^e, 2^18*bf16(ks*k))            (exact products)
    P2: fp8e4 DoubleRow matmul of the (q hi,lo) pair against the
        duplicated fp8 k-lo residual (2^18 split 2^8 * 2^10)
    P3: rank-6 bf16 correction (u/rho)*km + (b/rho)*c  (hi/lo split)
  row max m (DVE per-chunk reduce), nm = -rho'*m + ln16 + delta
  e16 = exp(rho'*sc + nm)  in [0, 16.003], accum_out -> z = 16*Z
  t = bf16(e16 + 127.5)    (Pool; bf16 round-to-nearest == floor+128)
  fT via PE transpose of t, eviction = relu(t^ - 128) (ACT/DVE split)
  PV = fT @ vd (bf16), out = PV * (1/z)

Exact in real arithmetic because for causal rows pmin=0, so
pd = floor(16*e)/(16*Z).  The single non-causal row (s=S-1) is computed
on the host. V dequant (v*vs+vm) is folded on the host into bf16 vd.
"""

import math
import os

import numpy as np
import ml_dtypes

S, B, H, D = 2048, 1, 16, 128
VG = 128
G = S // VG
P_LEVELS = 16.0
N_CORES = 8
HPC = H // N_CORES  # heads per core = 2
RB = 128            # row-block (query tile) size
NRB = S // RB       # 16 row-blocks
NKT = S // 128      # 16 key tiles

BF16 = ml_dtypes.bfloat16
FP8E4 = ml_dtypes.float8_e4m3
DELTA = 2e-4
LN16D = float(np.log(np.float64(16.0)) + DELTA)
GS = 18             # global log2 scale on the scores PSUM
QS8 = 8             # q-pair fp8 pre-scale (2^QS8)
KS8 = GS - QS8      # k-lo fp8 pre-scale

_COMPILED = None


def _build_graph():
    import concourse.bass as bass
    import concourse.bacc as bacc
    import concourse.tile as tile
    import concourse.mybir as mybir

    f32 = mybir.dt.float32
    bf16 = mybir.dt.bfloat16
    fp8e4 = mybir.dt.float8e4
    Alu = mybir.AluOpType
    Act = mybir.ActivationFunctionType

    nc = bacc.Bacc("TRN2", target_bir_lowering=False, debug=False,
                   num_devices=N_CORES)

    qT_d = nc.declare_dram_parameter("qT", [HPC, 128, S], bf16, isOutput=False)
    kTh_d = nc.declare_dram_parameter("kTh", [HPC, 128, S], bf16, isOutput=False)
    qp8_d = nc.declare_dram_parameter("qp8", [HPC, 128, 2, S], fp8e4,
                                      isOutput=False)
    kl8_d = nc.declare_dram_parameter("kl8", [HPC, 128, 2, S], fp8e4,
                                      isOutput=False)
    rho_d = nc.declare_dram_parameter("rho", [HPC, 128, NRB], f32, isOutput=False)
    nrho_d = nc.declare_dram_parameter("nrho", [HPC, 128, NRB], f32, isOutput=False)
    r2l_d = nc.declare_dram_parameter("r2l", [HPC, 6, S], bf16, isOutput=False)
    r2r_d = nc.declare_dram_parameter("r2r", [HPC, 6, S], bf16, isOutput=False)
    v_d = nc.declare_dram_parameter("vv", [HPC, 128, NKT, 128], bf16,
                                    isOutput=False)
    mask_d = nc.declare_dram_parameter("mask", [128, 128], f32, isOutput=False)
    n128_d = nc.declare_dram_parameter("neg128", [128, 1], f32, isOutput=False)
    id_d = nc.declare_dram_parameter("ident", [128, 128], bf16, isOutput=False)
    out_d = nc.declare_dram_parameter("out", [HPC, NRB, 128, 128], f32,
                                      isOutput=True)

    CHUNK = int(os.environ.get("K_CHUNK", 1024))
    PS_S = int(os.environ.get("K_PSS", 3))
    PS_T = int(os.environ.get("K_PST", 1))
    PS_V = int(os.environ.get("K_PSV", 1))
    WB = int(os.environ.get("K_WB", 3))
    TG = int(os.environ.get("K_TG", 8))          # tiles per transpose group
    EV_MOD = int(os.environ.get("K_EVM", 3))     # eviction: gi%EV_MOD==EV_ACT -> ACT
    EV_ACT = int(os.environ.get("K_EVA", 2))
    MG_ENG = os.environ.get("K_MG", "g")         # magic-add engine: g=Pool
    USE_FP8_P2 = int(os.environ.get("K_FP8P2", 1))

    with tile.TileContext(nc) as tc:
        with (
            tc.tile_pool(name="const", bufs=1) as constp,
            tc.tile_pool(name="heads", bufs=2) as headp,
            tc.tile_pool(name="work", bufs=WB) as workp,
            tc.tile_pool(name="stat", bufs=int(os.environ.get("K_SB", 6))) as statp,
            tc.tile_pool(name="ps_s", bufs=PS_S, space="PSUM") as ps_s,
            tc.tile_pool(name="ps_t", bufs=PS_T, space="PSUM") as ps_t,
            tc.tile_pool(name="ps_v", bufs=PS_V, space="PSUM") as ps_v,
        ):
            mask_sb = constp.tile([128, 128], f32, tag="mask")
            nc.sync.dma_start(mask_sb[:], mask_d[:])
            n128_sb = constp.tile([128, 1], f32, tag="neg128")
            nc.sync.dma_start(n128_sb[:], n128_d[:])
            id_sb = constp.tile([128, 128], bf16, tag="ident")
            nc.sync.dma_start(id_sb[:], id_d[:])

            gi = 0  # global transpose-group counter (eviction engine split)
            for h in range(HPC):
                qT_sb = headp.tile([128, S], bf16, tag="qT")
                nc.sync.dma_start(qT_sb[:], qT_d[h])
                kTh_sb = headp.tile([128, S], bf16, tag="kTh")
                nc.sync.dma_start(kTh_sb[:], kTh_d[h])
                if USE_FP8_P2:
                    qp8_sb = headp.tile([128, 2, S], fp8e4, tag="qp8")
                    nc.sync.dma_start(qp8_sb[:], qp8_d[h])
                    kl8_sb = headp.tile([128, 2, S], fp8e4, tag="kl8")
                    nc.sync.dma_start(kl8_sb[:], kl8_d[h])
                else:
                    kTl_sb = headp.tile([128, S], bf16, tag="kTl")
                    nc.sync.dma_start(kTl_sb[:], qp8_d[h])  # unused path
                rho_sb = headp.tile([128, NRB], f32, tag="rho")
                nc.sync.dma_start(rho_sb[:], rho_d[h])
                nrho_sb = headp.tile([128, NRB], f32, tag="nrho")
                nc.sync.dma_start(nrho_sb[:], nrho_d[h])
                r2l_sb = headp.tile([6, S], bf16, tag="r2l")
                nc.sync.dma_start(r2l_sb[:], r2l_d[h])
                r2r_sb = headp.tile([6, S], bf16, tag="r2r")
                nc.sync.dma_start(r2r_sb[:], r2r_d[h])
                v_sb = headp.tile([128, NKT, 128], bf16, tag="vv")
                nc.sync.dma_start(v_sb[:], v_d[h])

                for rb in range(NRB):
                    T = rb + 1
                    NK = T * 128
                    q0 = rb * 128
                    nch = (NK + CHUNK - 1) // CHUNK

                    # Per-chunk softmax: each chunk exponentiates against its
                    # own chunk max as soon as its matmuls+reduce are done
                    # (frees the PSUM bank early); a per-chunk rescale factor
                    # s1_c = exp(rho*m_c + nm) = 16*e^d*e^(rho*(m_c-m)) folds
                    # into the Pool magic multiply.
                    mx = statp.tile([128, 4], f32, tag="mx")
                    nm = statp.tile([128, 1], f32, tag="nm")
                    e = workp.tile([128, S], f32, tag="e")
                    zc = statp.tile([128, 4], f32, tag="zc")
                    for c in range(nch):
                        k0 = c * CHUNK
                        kn = min(NK, k0 + CHUNK) - k0
                        sc = ps_s.tile([128, CHUNK], f32, tag="sc")
                        for n0 in range(0, kn, 512):
                            n1 = min(kn, n0 + 512)
                            nc.tensor.matmul(sc[:, n0:n1],
                                             qT_sb[:, q0:q0 + 128],
                                             kTh_sb[:, k0 + n0:k0 + n1],
                                             start=True, stop=False)
                            if USE_FP8_P2:
                                nc.tensor.matmul(
                                    sc[:, n0:n1],
                                    qp8_sb[:, :, q0:q0 + 128],
                                    kl8_sb[:, :, k0 + n0:k0 + n1],
                                    start=False, stop=False,
                                    perf_mode=mybir.MatmulPerfMode.DoubleRow)
                            else:
                                nc.tensor.matmul(sc[:, n0:n1],
                                                 qT_sb[:, q0:q0 + 128],
                                                 kTl_sb[:, k0 + n0:k0 + n1],
                                                 start=False, stop=False)
                            nc.tensor.matmul(sc[:, n0:n1],
                                             r2l_sb[:, q0:q0 + 128],
                                             r2r_sb[:, k0 + n0:k0 + n1],
                                             start=False, stop=True)
                        if k0 <= q0 < k0 + kn:
                            d0 = q0 - k0
                            nc.vector.tensor_add(sc[:, d0:d0 + 128],
                                                 sc[:, d0:d0 + 128],
                                                 mask_sb[:])
                        nc.vector.tensor_reduce(mx[:, c:c + 1], sc[:, :kn],
                                                axis=mybir.AxisListType.X,
                                                op=Alu.max)
                        if nch > 1:
                            # chunk-local bias -rho*m_c (no ln16: that comes
                            # in via the s1 rescale)
                            nmc = statp.tile([128, 1], f32, tag="nmc")
                            nc.vector.tensor_scalar(nmc[:], mx[:, c:c + 1],
                                                    nrho_sb[:, rb:rb + 1],
                                                    None, Alu.mult)
                            bias_ap = nmc[:]
                        else:
                            nc.vector.tensor_scalar(nm[:], mx[:, 0:1],
                                                    nrho_sb[:, rb:rb + 1],
                                                    LN16D,
                                                    Alu.mult, Alu.add)
                            bias_ap = nm[:]
                        nc.scalar.activation(e[:, k0:k0 + kn], sc[:, :kn],
                                             Act.Exp,
                                             bias=bias_ap,
                                             scale=rho_sb[:, rb:rb + 1],
                                             accum_out=zc[:, c:c + 1])
                    if nch > 1:
                        m = statp.tile([128, 1], f32, tag="m")
                        nc.vector.tensor_reduce(m[:], mx[:, :nch],
                                                axis=mybir.AxisListType.X,
                                                op=Alu.max)
                        nc.vector.tensor_scalar(nm[:], m[:],
                                                nrho_sb[:, rb:rb + 1],
                                                LN16D,
                                                Alu.mult, Alu.add)

                    r = statp.tile([128, 1], f32, tag="r")
                    t = workp.tile([128, S], bf16, tag="t")
                    eng = nc.gpsimd if MG_ENG == "g" else nc.vector
                    if nch > 1:
                        # s1_c = exp(rho*m_c + nm); s1 of the winning chunk
                        # is exactly 16*e^d
                        s1 = statp.tile([128, 4], f32, tag="s1")
                        nc.scalar.activation(s1[:, :nch], mx[:, :nch],
                                             Act.Exp,
                                             bias=nm[:],
                                             scale=rho_sb[:, rb:rb + 1])
                        w = statp.tile([128, 4], f32, tag="w")
                        nc.vector.tensor_tensor(w[:, :nch], zc[:, :nch],
                                                s1[:, :nch], op=Alu.mult)
                        z = statp.tile([128, 1], f32, tag="z")
                        nc.vector.tensor_reduce(z[:], w[:, :nch],
                                                axis=mybir.AxisListType.X,
                                                op=Alu.add)
                        nc.vector.reciprocal(r[:], z[:])
                        for c in range(nch):
                            k0 = c * CHUNK
                            kn = min(NK, k0 + CHUNK) - k0
                            eng.tensor_scalar(t[:, k0:k0 + kn],
                                              e[:, k0:k0 + kn],
                                              s1[:, c:c + 1], 127.5,
                                              Alu.mult, Alu.add)
                    else:
                        nc.vector.reciprocal(r[:], zc[:, 0:1])
                        eng.tensor_scalar(t[:, :NK], e[:, :NK],
                                          127.5, None, Alu.add)

                    # transpose 128x128 tiles; evict with relu(t^ - 128)
                    fT = workp.tile([128, S], bf16, tag="fT")
                    for t0 in range(0, T, TG):
                        tn = min(TG, T - t0)
                        ptr = ps_t.tile([128, TG * 128], bf16, tag="tr")
                        for i in range(tn):
                            tt = t0 + i
                            nc.tensor.transpose(ptr[:, i * 128:(i + 1) * 128],
                                                t[:, tt * 128:(tt + 1) * 128],
                                                id_sb[:])
                        dst = fT[:, t0 * 128:(t0 + tn) * 128]
                        if gi % EV_MOD == EV_ACT:
                            nc.scalar.activation(dst, ptr[:, :tn * 128],
                                                 Act.Relu,
                                                 bias=n128_sb[:], scale=1.0)
                        else:
                            nc.vector.tensor_scalar(dst, ptr[:, :tn * 128],
                                                    128.0, 0.0,
                                                    Alu.subtract, Alu.max)
                        gi += 1

                    pv = ps_v.tile([128, 128], f32, tag="pv")
                    for tt in range(T):
                        nc.tensor.matmul(pv[:],
                                         fT[:, tt * 128:(tt + 1) * 128],
                                         v_sb[:, tt, :],
                                         start=(tt == 0), stop=(tt == T - 1))

                    o = workp.tile([128, 128], f32, tag="o")
                    if os.environ.get("K_OSC", "a") == "a":
                        nc.scalar.mul(o[:], pv[:], r[:])
                    else:
                        nc.vector.tensor_scalar(o[:], pv[:], r[:], None,
                                                Alu.mult)
                    nc.sync.dma_start(out_d[h, rb], o[:])

    nc.compile()
    return nc


def _host_prep(query, key, value, qmin, qscale, kmin, kscale, vmin, vscale):
    """Builds per-head device inputs, stacked [H, ...]."""
    f32 = np.float32
    q = query[:, 0, :, :].astype(f32)     # [S, H, D]
    k = key[:, 0, :, :].astype(f32)
    v = value[:, 0, :, :].astype(f32)
    qs = qscale[:, 0, :].astype(f32)      # [S, H]
    qm = qmin[:, 0, :].astype(f32)
    ks = kscale[:, 0, :].astype(f32)
    km = kmin[:, 0, :].astype(f32)
    vs = vscale[:, 0, :, :].astype(f32)   # [G, H, D]
    vm = vmin[:, 0, :, :].astype(f32)

    rsd = f32(1.0 / math.sqrt(D))
    a = qs * rsd
    b = qm * rsd
    sq = q.sum(axis=2)
    sk = k.sum(axis=2)
    u = a * sq + b * f32(D)
    c = ks * sk

    # q side: a = rho * 2^e; q2e = q * 2^e exact in bf16.
    e_i = np.round(np.log2(a))
    two_e = np.exp2(e_i).astype(f32)
    rho = (a / two_e).astype(f32)
    q2e = q * two_e[:, :, None]                         # [S, H, D] exact
    qT = np.ascontiguousarray(q2e.transpose(1, 2, 0)).astype(BF16)  # [H,D,S]

    # scores PSUM is globally scaled by 2^GS; rho' = rho * 2^-GS
    rho_s = (rho * f32(2.0 ** -GS)).astype(f32)

    # k hi: bf16(ks*k) * 2^GS (exact exponent shift after rounding)
    kp = (k * ks[:, :, None]).astype(f32)
    kph = kp.astype(BF16).astype(f32)
    kTh = np.ascontiguousarray((kph * f32(2.0 ** GS)).transpose(1, 2, 0)
                               ).astype(BF16)           # [H, D, S]

    # k lo residual as fp8 pair path: q-pair (hi/lo nibbles of q2e * 2^QS8)
    q2e_s = (q2e * f32(2.0 ** QS8)).astype(f32)
    q1 = q2e_s.astype(FP8E4).astype(f32)
    q2 = (q2e_s - q1).astype(FP8E4)
    qp8 = np.stack([q1.astype(FP8E4), q2], axis=2)      # [S, H, 2, D]
    qp8 = np.ascontiguousarray(qp8.transpose(1, 3, 2, 0))  # [H, D, 2, S]

    kpl = ((kp - kph) * f32(2.0 ** KS8)).astype(FP8E4)  # [S, H, D]
    kl8 = np.stack([kpl, kpl], axis=2)                  # [S, H, 2, D]
    kl8 = np.ascontiguousarray(kl8.transpose(1, 3, 2, 0))  # [H, D, 2, S]

    rho_r = np.ascontiguousarray(
        rho_s.T.reshape(H, NRB, 128).transpose(0, 2, 1)).astype(f32)
    nrho_r = np.ascontiguousarray(-rho_r)

    def hilo(x):
        xh = x.astype(BF16).astype(f32)
        xl = (x - xh).astype(BF16).astype(f32)
        return xh, xl

    # rank-6 correction, scaled 2^GS split as 2^9 per side
    s9 = f32(2.0 ** 9)
    up = (u / rho * s9).astype(f32)
    bp = (b / rho * s9).astype(f32)
    kms = (km * s9).astype(f32)
    cs = (c * s9).astype(f32)
    uh, ul = hilo(up)
    bh, bl = hilo(bp)
    kmh, kml = hilo(kms)
    ch, cl = hilo(cs)
    r2l = np.stack([uh, uh, ul, bh, bh, bl], axis=0)     # [6, S, H]
    r2r = np.stack([kmh, kml, kmh, ch, cl, ch], axis=0)
    r2l = np.ascontiguousarray(r2l.transpose(2, 0, 1)).astype(BF16)  # [H,6,S]
    r2r = np.ascontiguousarray(r2r.transpose(2, 0, 1)).astype(BF16)

    vs_full = np.repeat(vs, VG, axis=0)
    vm_full = np.repeat(vm, VG, axis=0)
    vd = v * vs_full + vm_full            # f32 [S, H, D]
    vdt = vd.transpose(1, 0, 2).reshape(H, NKT, 128, D)
    vdt = np.ascontiguousarray(vdt.transpose(0, 2, 1, 3)).astype(BF16)

    mask = np.triu(np.full((128, 128), -1e30, dtype=f32), k=1)
    ident = np.eye(128, dtype=np.float32).astype(BF16)

    return dict(qT=qT, kTh=kTh, qp8=qp8, kl8=kl8, rho=rho_r, nrho=nrho_r,
                r2l=r2l, r2r=r2r, vv=vdt, mask=mask,
                neg128=np.full((128, 1), -128.0, dtype=f32),
                ident=ident, vd_f32=vd)


def _host_last_row(query, key, qmin, qscale, kmin, kscale, vd_f32):
    """Exact reference math (numpy f32) for the single non-causal row."""
    f32 = np.float32
    i = S - 1
    out = np.zeros((H, D), dtype=f32)
    for h in range(H):
        qd = query[i, 0, h, :].astype(f32) * f32(qscale[i, 0, h]) + f32(qmin[i, 0, h])
        kd = key[:, 0, h, :].astype(f32) * kscale[:, 0, h].astype(f32)[:, None] \
            + kmin[:, 0, h].astype(f32)[:, None]
        s = (kd @ qd).astype(f32) * f32(1.0 / math.sqrt(D))
        e = np.exp(s - s.max(), dtype=f32)
        p = (e / e.sum(dtype=f32)).astype(f32)
        pmax, pmin_ = p.max(), p.min()
        pscale = (pmax - pmin_) / f32(P_LEVELS)
        safe = pscale if pscale > 0 else f32(1.0)
        pq = np.floor((p - pmin_) / safe).astype(f32)
        pd = pq * pscale + pmin_
        out[h] = pd @ vd_f32[:, h, :]
    return out


def _reference_numpy(query, key, value, qmin, qscale, kmin, kscale,
                     vmin, vscale, causal):
    f32 = np.float32
    q = query[:, 0, :, :].astype(f32)
    k = key[:, 0, :, :].astype(f32)
    v = value[:, 0, :, :].astype(f32)
    out = np.zeros((S, B, H * D), dtype=f32)
    vs_full = np.repeat(vscale[:, 0, :, :].astype(f32), VG, axis=0)
    vm_full = np.repeat(vmin[:, 0, :, :].astype(f32), VG, axis=0)
    for h in range(H):
        qd = q[:, h, :] * qscale[:, 0, h].astype(f32)[:, None] + qmin[:, 0, h].astype(f32)[:, None]
        kd = k[:, h, :] * kscale[:, 0, h].astype(f32)[:, None] + kmin[:, 0, h].astype(f32)[:, None]
        s = (qd @ kd.T) * f32(1.0 / math.sqrt(D))
        if causal:
            s = np.where(np.tril(np.ones((S, S), dtype=bool)), s, f32(-1e30))
        e = np.exp(s - s.max(axis=1, keepdims=True), dtype=f32)
        p = e / e.sum(axis=1, keepdims=True, dtype=f32)
        pmax = p.max(axis=1, keepdims=True)
        pmin_ = p.min(axis=1, keepdims=True)
        pscale = (pmax - pmin_) / f32(P_LEVELS)
        safe = np.where(pscale > 0, pscale, f32(1.0))
        pd = np.floor((p - pmin_) / safe) * pscale + pmin_
        vd = v[:, h, :] * vs_full[:, h, :] + vm_full[:, h, :]
        out[:, 0, h * D:(h + 1) * D] = pd.astype(f32) @ vd
    return out


def kernel(query, key, value, qmin, qscale, kmin, kscale, vmin, vscale,
           causal):
    global _COMPILED
    causal_i = int(np.asarray(causal))
    if causal_i != 1:
        return _reference_numpy(query, key, value, qmin, qscale, kmin,
                                kscale, vmin, vscale, causal_i)

    prep = _host_prep(query, key, value, qmin, qscale, kmin, kscale,
                      vmin, vscale)

    if _COMPILED is None:
        _COMPILED = _build_graph()
    nc = _COMPILED

    in_maps = []
    for core in range(N_CORES):
        hs = slice(core * HPC, (core + 1) * HPC)
        in_maps.append({
            "qT": np.ascontiguousarray(prep["qT"][hs]),
            "kTh": np.ascontiguousarray(prep["kTh"][hs]),
            "qp8": np.ascontiguousarray(prep["qp8"][hs]),
            "kl8": np.ascontiguousarray(prep["kl8"][hs]),
            "rho": np.ascontiguousarray(prep["rho"][hs]),
            "nrho": np.ascontiguousarray(prep["nrho"][hs]),
            "r2l": np.ascontiguousarray(prep["r2l"][hs]),
            "r2r": np.ascontiguousarray(prep["r2r"][hs]),
            "vv": np.ascontiguousarray(prep["vv"][hs]),
            "mask": prep["mask"],
            "neg128": prep["neg128"],
            "ident": prep["ident"],
        })

    from concourse.bass_utils import run_bass_kernel_spmd
    trace = bool(int(os.environ.get("KERNEL_TRACE", "0")))
    res = run_bass_kernel_spmd(nc, in_maps, core_ids=list(range(N_CORES)),
                               trace=trace)
    if res.exec_time_ns is not None:
        kernel.last_exec_ns = res.exec_time_ns
        print(f"HW exec time: {res.exec_time_ns} ns")

    out = np.zeros((S, B, H * D), dtype=np.float32)
    for core in range(N_CORES):
        o = np.asarray(res.results[core]["out"], dtype=np.float32)
        for j in range(HPC):
            h = core * HPC + j
            out[:, 0, h * D:(h + 1) * D] = o[j].reshape(S, D)

    last = _host_last_row(query, key, qmin, qscale, kmin, kscale,
                          prep["vd_f32"])
    for h in range(H):
        out[S - 1, 0, h * D:(h + 1) * D] = last[h]
    return out


kernel.last_exec_ns = None


# revision 45
# speedup vs baseline: 1.3488x; 1.0364x over previous
"""
Sparse (quantized) attention on 8 Trainium2 NeuronCores.

Strategy: head-parallel sharding. 16 (b,h) heads -> 2 heads per core, no
collectives. Per head the device computes, for each 128-query row-block
(causal: only the first rb+1 key tiles):

  scores*2^18 in PSUM via
    P1: bf16-matmul(q*2^e, 2^18*bf16(ks*k))            (exact products)
    P2: fp8e4 DoubleRow matmul of the (q hi,lo) pair against the
        duplicated fp8 k-lo residual (2^18 split 2^8 * 2^10)
    P3: rank-6 bf16 correction (u/rho)*km + (b/rho)*c  (hi/lo split)
  row max m (DVE per-chunk reduce), nm = -rho'*m + ln16 + delta
  e16 = exp(rho'*sc + nm)  in [0, 16.003], accum_out -> z = 16*Z
  t = bf16(e16 + 127.5)    (Pool; bf16 round-to-nearest == floor+128)
  fT via PE transpose of t, eviction = relu(t^ - 128) (ACT/DVE split)
  PV = fT @ vd (bf16), out = PV * (1/z)

Exact in real arithmetic because for causal rows pmin=0, so
pd = floor(16*e)/(16*Z).  The single non-causal row (s=S-1) is computed
on the host. V dequant (v*vs+vm) is folded on the host into bf16 vd.
"""

import math
import os

import numpy as np
import ml_dtypes

S, B, H, D = 2048, 1, 16, 128
VG = 128
G = S // VG
P_LEVELS = 16.0
N_CORES = 8
HPC = H // N_CORES  # heads per core = 2
RB = 128            # row-block (query tile) size
NRB = S // RB       # 16 row-blocks
NKT = S // 128      # 16 key tiles

BF16 = ml_dtypes.bfloat16
FP8E4 = ml_dtypes.float8_e4m3
FP8E5 = ml_dtypes.float8_e5m2
LMAX = 6            # corr fp8e5 level pairs (i+j <= LMAX)
NLV = LMAX + 1
PAIRS = [(i, j) for i in range(NLV) for j in range(NLV) if i + j <= LMAX]
NPAIR = len(PAIRS)  # 28 cells, 2 products per cell
DELTA = 2e-4
LN16D = float(np.log(np.float64(16.0)) + DELTA)
GS = 18             # global log2 scale on the scores PSUM
QS8 = 8             # q-pair fp8 pre-scale (2^QS8)
KS8 = GS - QS8      # k-lo fp8 pre-scale

_COMPILED = None


def _build_graph():
    import concourse.bass as bass
    import concourse.bacc as bacc
    import concourse.tile as tile
    import concourse.mybir as mybir

    f32 = mybir.dt.float32
    bf16 = mybir.dt.bfloat16
    fp8e4 = mybir.dt.float8e4
    fp8e5 = mybir.dt.float8e5
    Alu = mybir.AluOpType
    Act = mybir.ActivationFunctionType

    nc = bacc.Bacc("TRN2", target_bir_lowering=False, debug=False,
                   num_devices=N_CORES)

    qT_d = nc.declare_dram_parameter("qT", [HPC, 128, S], bf16, isOutput=False)
    kTh_d = nc.declare_dram_parameter("kTh", [HPC, 128, S], bf16, isOutput=False)
    qp8_d = nc.declare_dram_parameter("qp8", [HPC, 128, 2, S], fp8e4,
                                      isOutput=False)
    kl8_d = nc.declare_dram_parameter("kl8", [HPC, 128, 2, S], fp8e4,
                                      isOutput=False)
    rho_d = nc.declare_dram_parameter("rho", [HPC, 128, NRB], f32, isOutput=False)
    nrho_d = nc.declare_dram_parameter("nrho", [HPC, 128, NRB], f32, isOutput=False)
    r2l_d = nc.declare_dram_parameter("r2l", [HPC, 6, S], bf16, isOutput=False)
    r2r_d = nc.declare_dram_parameter("r2r", [HPC, 6, S], bf16, isOutput=False)
    r3l_d = nc.declare_dram_parameter("r3l", [HPC, NPAIR, 2, S], fp8e5,
                                      isOutput=False)
    r3r_d = nc.declare_dram_parameter("r3r", [HPC, NPAIR, 2, S], fp8e5,
                                      isOutput=False)
    v_d = nc.declare_dram_parameter("vv", [HPC, 128, NKT, 128], bf16,
                                    isOutput=False)
    mask_d = nc.declare_dram_parameter("mask", [128, 128], f32, isOutput=False)
    n128_d = nc.declare_dram_parameter("neg128", [128, 1], f32, isOutput=False)
    id_d = nc.declare_dram_parameter("ident", [128, 128], bf16, isOutput=False)
    out_d = nc.declare_dram_parameter("out", [HPC, NRB, 128, 128], f32,
                                      isOutput=True)

    CHUNK = int(os.environ.get("K_CHUNK", 1024))
    TRMODE = os.environ.get("K_TR", "pe")        # dma | pe transpose path
    PS_S = int(os.environ.get("K_PSS", 3))
    PS_T = int(os.environ.get("K_PST", 1 if TRMODE == "pe" else 0))
    PS_V = int(os.environ.get("K_PSV", 1 if TRMODE == "pe" else 2))
    WB = int(os.environ.get("K_WB", 4))
    TG = int(os.environ.get("K_TG", 8))          # tiles per transpose group
    EV_MOD = int(os.environ.get("K_EVM", 3))     # eviction: gi%EV_MOD==EV_ACT -> ACT
    EV_ACT = int(os.environ.get("K_EVA", 2))
    MG_ENG = os.environ.get("K_MG", "g")         # magic-add engine: g=Pool
    RL_ENG = os.environ.get("K_RL", "v")         # relu engine (dma path)
    USE_FP8_P2 = int(os.environ.get("K_FP8P2", 1))
    USE_FP8_P3 = int(os.environ.get("K_FP8P3", 1))

    import contextlib
    with tile.TileContext(nc) as tc:
        with contextlib.ExitStack() as es:
            constp = es.enter_context(tc.tile_pool(name="const", bufs=1))
            headp = es.enter_context(tc.tile_pool(name="heads", bufs=2))
            workp = es.enter_context(tc.tile_pool(name="work", bufs=WB))
            statp = es.enter_context(
                tc.tile_pool(name="stat", bufs=int(os.environ.get("K_SB", 6))))
            ps_s = es.enter_context(
                tc.tile_pool(name="ps_s", bufs=PS_S, space="PSUM"))
            ps_v = es.enter_context(
                tc.tile_pool(name="ps_v", bufs=PS_V, space="PSUM"))
            ps_t = (es.enter_context(
                tc.tile_pool(name="ps_t", bufs=PS_T, space="PSUM"))
                if PS_T > 0 else None)
            mask_sb = constp.tile([128, 128], f32, tag="mask")
            nc.sync.dma_start(mask_sb[:], mask_d[:])
            n128_sb = constp.tile([128, 1], f32, tag="neg128")
            nc.sync.dma_start(n128_sb[:], n128_d[:])
            id_sb = constp.tile([128, 128], bf16, tag="ident")
            nc.sync.dma_start(id_sb[:], id_d[:])
            # warm the ACT exp table so LoadActFuncSet is off the critical path
            warm = constp.tile([128, 1], f32, tag="warm")
            nc.gpsimd.memset(warm[:], 0.0)
            nc.scalar.activation(warm[:], warm[:], Act.Exp)

            gi = 0  # global transpose-group counter (eviction engine split)
            SPL = int(os.environ.get("K_SPL", 768))  # first-piece columns
            ILV = int(os.environ.get("K_ILV", 0))    # interleave the 2 heads
            hdat = []
            for h in range(HPC):
                d = {}
                d["qT"] = headp.tile([128, S], bf16, tag="qT", name=f"qT{h}")
                d["kTh"] = headp.tile([128, S], bf16, tag="kTh", name=f"kTh{h}")
                if USE_FP8_P2:
                    d["qp8"] = headp.tile([128, 2, S], fp8e4, tag="qp8", name=f"qp8{h}")
                    d["kl8"] = headp.tile([128, 2, S], fp8e4, tag="kl8", name=f"kl8{h}")
                else:
                    d["kTl"] = headp.tile([128, S], bf16, tag="kTl", name=f"kTl{h}")
                d["rho"] = headp.tile([128, NRB], f32, tag="rho", name=f"rho{h}")
                d["nrho"] = headp.tile([128, NRB], f32, tag="nrho", name=f"nrho{h}")
                if USE_FP8_P3:
                    d["r2l"] = headp.tile([NPAIR, 2, S], fp8e5, tag="r2l",
                                          name=f"r3l{h}")
                    d["r2r"] = headp.tile([NPAIR, 2, S], fp8e5, tag="r2r",
                                          name=f"r3r{h}")
                else:
                    d["r2l"] = headp.tile([6, S], bf16, tag="r2l", name=f"r2l{h}")
                    d["r2r"] = headp.tile([6, S], bf16, tag="r2r", name=f"r2r{h}")
                d["v"] = headp.tile([128, NKT, 128], bf16, tag="vv", name=f"vv{h}")
                hdat.append(d)
            # stage the first SPL columns of the score operands (both heads)
            # so the PE can start while the bulk still streams in
            for h in range(HPC):
                spl = SPL if (h == 0 or ILV) and SPL > 0 else 0
                d = hdat[h]
                if spl:
                    nc.sync.dma_start(d["qT"][:, :spl], qT_d[h][:, :spl])
                    nc.sync.dma_start(d["kTh"][:, :spl], kTh_d[h][:, :spl])
                    if USE_FP8_P2:
                        nc.sync.dma_start(d["qp8"][:, :, :spl],
                                          qp8_d[h][:, :, :spl])
                        nc.sync.dma_start(d["kl8"][:, :, :spl],
                                          kl8_d[h][:, :, :spl])
                    if USE_FP8_P3:
                        nc.sync.dma_start(d["r2l"][:, :, :spl],
                                          r3l_d[h][:, :, :spl])
                        nc.sync.dma_start(d["r2r"][:, :, :spl],
                                          r3r_d[h][:, :, :spl])
                    else:
                        nc.sync.dma_start(d["r2l"][:, :spl], r2l_d[h][:, :spl])
                        nc.sync.dma_start(d["r2r"][:, :spl], r2r_d[h][:, :spl])
            for h in range(HPC):
                spl = SPL if (h == 0 or ILV) and SPL > 0 else 0
                d = hdat[h]
                nc.sync.dma_start(d["rho"][:], rho_d[h])
                nc.sync.dma_start(d["nrho"][:], nrho_d[h])
                if spl:
                    nc.sync.dma_start(d["qT"][:, spl:], qT_d[h][:, spl:])
                    nc.sync.dma_start(d["kTh"][:, spl:], kTh_d[h][:, spl:])
                    if USE_FP8_P2:
                        nc.sync.dma_start(d["qp8"][:, :, spl:],
                                          qp8_d[h][:, :, spl:])
                        nc.sync.dma_start(d["kl8"][:, :, spl:],
                                          kl8_d[h][:, :, spl:])
                    if USE_FP8_P3:
                        nc.sync.dma_start(d["r2l"][:, :, spl:],
                                          r3l_d[h][:, :, spl:])
                        nc.sync.dma_start(d["r2r"][:, :, spl:],
                                          r3r_d[h][:, :, spl:])
                    else:
                        nc.sync.dma_start(d["r2l"][:, spl:], r2l_d[h][:, spl:])
                        nc.sync.dma_start(d["r2r"][:, spl:], r2r_d[h][:, spl:])
                else:
                    nc.sync.dma_start(d["qT"][:], qT_d[h])
                    nc.sync.dma_start(d["kTh"][:], kTh_d[h])
                    if USE_FP8_P2:
                        nc.sync.dma_start(d["qp8"][:], qp8_d[h])
                        nc.sync.dma_start(d["kl8"][:], kl8_d[h])
                    if USE_FP8_P3:
                        nc.sync.dma_start(d["r2l"][:], r3l_d[h])
                        nc.sync.dma_start(d["r2r"][:], r3r_d[h])
                    else:
                        nc.sync.dma_start(d["r2l"][:], r2l_d[h])
                        nc.sync.dma_start(d["r2r"][:], r2r_d[h])
                if not USE_FP8_P2:
                    nc.sync.dma_start(d["kTl"][:], qp8_d[h])  # unused path
                nc.sync.dma_start(d["v"][:], v_d[h])

            if ILV:
                order = [(it % HPC, it // HPC) for it in range(HPC * NRB)]
            else:
                order = [(h, rb) for h in range(HPC) for rb in range(NRB)]

            def stage1(h, rb):
                    d = hdat[h]
                    qT_sb = d["qT"]
                    kTh_sb = d["kTh"]
                    if USE_FP8_P2:
                        qp8_sb = d["qp8"]
                        kl8_sb = d["kl8"]
                    else:
                        kTl_sb = d["kTl"]
                    rho_sb = d["rho"]
                    nrho_sb = d["nrho"]
                    r2l_sb = d["r2l"]
                    r2r_sb = d["r2r"]
                    T = rb + 1
                    NK = T * 128
                    q0 = rb * 128
                    nch = (NK + CHUNK - 1) // CHUNK

                    # Rescale groups: each non-diagonal chunk-rest plus the
                    # 128-col diagonal tile exponentiate against their own
                    # local max as soon as their matmuls (+fused mask/max for
                    # the diagonal, via tensor_tensor_reduce) complete; the
                    # per-group factor s1_g = exp(rho*m_g + nm) folds into
                    # the Pool magic multiply.  Frees PSUM banks early and
                    # takes the diagonal mask off the big chunk's chain.
                    mx = statp.tile([128, 6], f32, tag="mx")
                    nm = statp.tile([128, 1], f32, tag="nm")
                    e = workp.tile([128, S], f32, tag="e")
                    zc = statp.tile([128, 6], f32, tag="zc")
                    groups = []  # (sc_tile, k0 global, off in tile, kn, diag)
                    for c in range(nch):
                        k0 = c * CHUNK
                        kn = min(NK, k0 + CHUNK) - k0
                        sc = ps_s.tile([128, CHUNK], f32, tag="sc")
                        RSP = int(os.environ.get("K_RSP", 0))
                        split = RSP and kn > 512
                        for n0 in range(0, kn, 512):
                            if n0 == 512 and split:
                                # first-half max overlaps second-half matmuls
                                nc.vector.tensor_reduce(
                                    mx[:, 4:5], sc[:, :512],
                                    axis=mybir.AxisListType.X, op=Alu.max)
                            n1 = min(kn, n0 + 512)
                            nc.tensor.matmul(sc[:, n0:n1],
                                             qT_sb[:, q0:q0 + 128],
                                             kTh_sb[:, k0 + n0:k0 + n1],
                                             start=True, stop=False)
                            if USE_FP8_P2:
                                nc.tensor.matmul(
                                    sc[:, n0:n1],
                                    qp8_sb[:, :, q0:q0 + 128],
                                    kl8_sb[:, :, k0 + n0:k0 + n1],
                                    start=False, stop=False,
                                    perf_mode=mybir.MatmulPerfMode.DoubleRow)
                            else:
                                nc.tensor.matmul(sc[:, n0:n1],
                                                 qT_sb[:, q0:q0 + 128],
                                                 kTl_sb[:, k0 + n0:k0 + n1],
                                                 start=False, stop=False)
                            if USE_FP8_P3:
                                nc.tensor.matmul(
                                    sc[:, n0:n1],
                                    r2l_sb[:, :, q0:q0 + 128],
                                    r2r_sb[:, :, k0 + n0:k0 + n1],
                                    start=False, stop=True,
                                    perf_mode=mybir.MatmulPerfMode.DoubleRow)
                            else:
                                nc.tensor.matmul(sc[:, n0:n1],
                                                 r2l_sb[:, q0:q0 + 128],
                                                 r2r_sb[:, k0 + n0:k0 + n1],
                                                 start=False, stop=True)
                        if c == nch - 1:
                            nc.vector.tensor_add(sc[:, kn - 128:kn],
                                                 sc[:, kn - 128:kn],
                                                 mask_sb[:])
                        if split:
                            nc.vector.tensor_reduce(
                                mx[:, 5:6], sc[:, 512:kn],
                                axis=mybir.AxisListType.X, op=Alu.max)
                            nc.vector.tensor_reduce(
                                mx[:, c:c + 1], mx[:, 4:6],
                                axis=mybir.AxisListType.X, op=Alu.max)
                        else:
                            nc.vector.tensor_reduce(mx[:, c:c + 1],
                                                    sc[:, :kn],
                                                    axis=mybir.AxisListType.X,
                                                    op=Alu.max)
                        groups.append((sc, k0, 0, kn, False))
                        if nch > 1:
                            nmc = statp.tile([128, 1], f32, tag="nmc")
                            nc.vector.tensor_scalar(nmc[:], mx[:, c:c + 1],
                                                    nrho_sb[:, rb:rb + 1],
                                                    None, Alu.mult)
                            bias_ap = nmc[:]
                        else:
                            nc.vector.tensor_scalar(nm[:], mx[:, 0:1],
                                                    nrho_sb[:, rb:rb + 1],
                                                    LN16D,
                                                    Alu.mult, Alu.add)
                            bias_ap = nm[:]
                        nc.scalar.activation(e[:, k0:k0 + kn],
                                             sc[:, :kn],
                                             Act.Exp,
                                             bias=bias_ap,
                                             scale=rho_sb[:, rb:rb + 1],
                                             accum_out=zc[:, c:c + 1])
                    G = len(groups)
                    if G > 1:
                        m = statp.tile([128, 1], f32, tag="m")
                        nc.vector.tensor_reduce(m[:], mx[:, :G],
                                                axis=mybir.AxisListType.X,
                                                op=Alu.max)
                        nc.vector.tensor_scalar(nm[:], m[:],
                                                nrho_sb[:, rb:rb + 1],
                                                LN16D,
                                                Alu.mult, Alu.add)

                    r = statp.tile([128, 1], f32, tag="r")
                    t = workp.tile([128, S], bf16, tag="t")
                    eng = nc.gpsimd if MG_ENG == "g" else nc.vector
                    MGW = int(os.environ.get("K_MGW", 2048))  # magic op width
                    if G > 1:
                        # s1_g = exp(rho*m_g + nm); winner = exactly 16*e^d
                        s1 = statp.tile([128, 6], f32, tag="s1")
                        nc.scalar.activation(s1[:, :G], mx[:, :G],
                                             Act.Exp,
                                             bias=nm[:],
                                             scale=rho_sb[:, rb:rb + 1])
                        w = statp.tile([128, 6], f32, tag="w")
                        nc.vector.tensor_tensor(w[:, :G], zc[:, :G],
                                                s1[:, :G], op=Alu.mult)
                        z = statp.tile([128, 1], f32, tag="z")
                        nc.vector.tensor_reduce(z[:], w[:, :G],
                                                axis=mybir.AxisListType.X,
                                                op=Alu.add)
                        nc.vector.reciprocal(r[:], z[:])
                        for g, (sc, k0, off, kn, diag) in enumerate(groups):
                            for m0 in range(0, kn, MGW):
                                m1 = min(kn, m0 + MGW)
                                eng.tensor_scalar(t[:, k0 + m0:k0 + m1],
                                                  e[:, k0 + m0:k0 + m1],
                                                  s1[:, g:g + 1], 127.5,
                                                  Alu.mult, Alu.add)
                    else:
                        nc.vector.reciprocal(r[:], zc[:, 0:1])
                        for m0 in range(0, NK, MGW):
                            m1 = min(NK, m0 + MGW)
                            eng.tensor_scalar(t[:, m0:m1], e[:, m0:m1],
                                              127.5, None, Alu.add)
                    return dict(t=t, r=r, groups=groups)

            def stage2(h, rb, ctx):
                    nonlocal gi
                    d = hdat[h]
                    v_sb = d["v"]
                    t = ctx["t"]
                    r = ctx["r"]
                    groups = ctx["groups"]
                    T = rb + 1
                    NK = T * 128

                    fT = workp.tile([128, NKT, 128], bf16, tag="fT")
                    if TRMODE == "dma":
                        # f = relu(t - 128) in SBUF (DVE 4x bf16), then the
                        # DMA crossbar transposes all T tiles in one shot
                        f = workp.tile([128, S], bf16, tag="f")
                        for g, (sc, k0, off, kn, diag) in enumerate(groups):
                            reng = nc.vector if RL_ENG == "v" else nc.gpsimd
                            reng.tensor_scalar(f[:, k0:k0 + kn],
                                               t[:, k0:k0 + kn],
                                               128.0, 0.0,
                                               Alu.subtract, Alu.max)
                        nc.sync.dma_start_transpose(fT[:, :T, :], f[:, :NK])
                    else:
                        for t0 in range(0, T, TG):
                            tn = min(TG, T - t0)
                            ptr = ps_t.tile([128, TG * 128], bf16, tag="tr")
                            for i in range(tn):
                                tt = t0 + i
                                nc.tensor.transpose(
                                    ptr[:, i * 128:(i + 1) * 128],
                                    t[:, tt * 128:(tt + 1) * 128],
                                    id_sb[:])
                            dst = fT[:, t0:t0 + tn, :]
                            if gi % EV_MOD == EV_ACT:
                                nc.scalar.activation(dst, ptr[:, :tn * 128],
                                                     Act.Relu,
                                                     bias=n128_sb[:],
                                                     scale=1.0)
                            else:
                                nc.vector.tensor_scalar(dst,
                                                        ptr[:, :tn * 128],
                                                        128.0, 0.0,
                                                        Alu.subtract, Alu.max)
                            gi += 1

                    pv = ps_v.tile([128, 128], f32, tag="pv")
                    for tt in range(T):
                        nc.tensor.matmul(pv[:],
                                         fT[:, tt, :],
                                         v_sb[:, tt, :],
                                         start=(tt == 0), stop=(tt == T - 1))

                    o = workp.tile([128, 128], f32, tag="o")
                    if os.environ.get("K_OSC", "a") == "a":
                        nc.scalar.mul(o[:], pv[:], r[:])
                    else:
                        nc.vector.tensor_scalar(o[:], pv[:], r[:], None,
                                                Alu.mult)
                    nc.sync.dma_start(out_d[h, rb], o[:])

            # software pipeline: emit stage1 of upcoming row-blocks before
            # stage2 of earlier ones so ready reduces/exps aren't queued
            # behind evictions that wait on the magic->transpose chain
            PIPE = int(os.environ.get("K_PIPE", 2))
            pend = []
            for (h, rb) in order:
                ctx = stage1(h, rb)
                pend.append((h, rb, ctx))
                if len(pend) >= PIPE:
                    h2, rb2, c2 = pend.pop(0)
                    stage2(h2, rb2, c2)
            for h2, rb2, c2 in pend:
                stage2(h2, rb2, c2)

    nc.compile()
    return nc


def _host_prep(query, key, value, qmin, qscale, kmin, kscale, vmin, vscale):
    """Builds per-head device inputs, stacked [H, ...]."""
    f32 = np.float32
    q = query[:, 0, :, :].astype(f32)     # [S, H, D]
    k = key[:, 0, :, :].astype(f32)
    v = value[:, 0, :, :].astype(f32)
    qs = qscale[:, 0, :].astype(f32)      # [S, H]
    qm = qmin[:, 0, :].astype(f32)
    ks = kscale[:, 0, :].astype(f32)
    km = kmin[:, 0, :].astype(f32)
    vs = vscale[:, 0, :, :].astype(f32)   # [G, H, D]
    vm = vmin[:, 0, :, :].astype(f32)

    rsd = f32(1.0 / math.sqrt(D))
    a = qs * rsd
    b = qm * rsd
    sq = q.sum(axis=2)
    sk = k.sum(axis=2)
    u = a * sq + b * f32(D)
    c = ks * sk

    # q side: a = rho * 2^e; q2e = q * 2^e exact in bf16.
    e_i = np.round(np.log2(a))
    two_e = np.exp2(e_i).astype(f32)
    rho = (a / two_e).astype(f32)
    q2e = q * two_e[:, :, None]                         # [S, H, D] exact
    qT = np.ascontiguousarray(q2e.transpose(1, 2, 0)).astype(BF16)  # [H,D,S]

    # scores PSUM is globally scaled by 2^GS; rho' = rho * 2^-GS
    rho_s = (rho * f32(2.0 ** -GS)).astype(f32)

    # k hi: bf16(ks*k) * 2^GS (exact exponent shift after rounding)
    kp = (k * ks[:, :, None]).astype(f32)
    kph = kp.astype(BF16).astype(f32)
    kTh = np.ascontiguousarray((kph * f32(2.0 ** GS)).transpose(1, 2, 0)
                               ).astype(BF16)           # [H, D, S]

    # k lo residual as fp8 pair path: q-pair (hi/lo nibbles of q2e * 2^QS8)
    q2e_s = (q2e * f32(2.0 ** QS8)).astype(f32)
    q1 = q2e_s.astype(FP8E4).astype(f32)
    q2 = (q2e_s - q1).astype(FP8E4)
    qp8 = np.stack([q1.astype(FP8E4), q2], axis=2)      # [S, H, 2, D]
    qp8 = np.ascontiguousarray(qp8.transpose(1, 3, 2, 0))  # [H, D, 2, S]

    kpl = ((kp - kph) * f32(2.0 ** KS8)).astype(FP8E4)  # [S, H, D]
    kl8 = np.stack([kpl, kpl], axis=2)                  # [S, H, 2, D]
    kl8 = np.ascontiguousarray(kl8.transpose(1, 3, 2, 0))  # [H, D, 2, S]

    rho_r = np.ascontiguousarray(
        rho_s.T.reshape(H, NRB, 128).transpose(0, 2, 1)).astype(f32)
    nrho_r = np.ascontiguousarray(-rho_r)

    def hilo(x):
        xh = x.astype(BF16).astype(f32)
        xl = (x - xh).astype(BF16).astype(f32)
        return xh, xl

    # rank-6 correction, scaled 2^GS split as 2^9 per side
    s9 = f32(2.0 ** 9)
    up = (u / rho).astype(f32)
    bp = (b / rho).astype(f32)
    uh, ul = hilo(up * s9)
    bh, bl = hilo(bp * s9)
    kmh, kml = hilo(km * s9)
    ch, cl = hilo(c * s9)
    r2l = np.stack([uh, uh, ul, bh, bh, bl], axis=0)     # [6, S, H]
    r2r = np.stack([kmh, kml, kmh, ch, cl, ch], axis=0)
    r2l = np.ascontiguousarray(r2l.transpose(2, 0, 1)).astype(BF16)  # [H,6,S]
    r2r = np.ascontiguousarray(r2r.transpose(2, 0, 1)).astype(BF16)

    # corr as fp8e5 DoubleRow level pairs: corr*2^GS = sum over PAIRS of
    # (upL_i*2^a)(kmL_j*2^(GS-a)) + (bpL_i*2^a)(cL_j*2^(GS-a))
    def e5_levels(x):
        parts = []
        rr = x.astype(f32).copy()
        for _ in range(NLV):
            mmx = max(float(np.abs(rr).max()), 1e-30)
            sh = f32(2.0 ** np.floor(np.log2(28672.0 / mmx)))
            p = (rr * sh).astype(FP8E5).astype(f32) / sh
            parts.append(p)
            rr = rr - p
        return parts

    upL = e5_levels(up)
    bpL = e5_levels(bp)
    kmL = e5_levels(km)
    cL = e5_levels(c)

    def pair_rows(lv_list, rv_list):
        lrows = np.zeros((NPAIR, S, H), dtype=FP8E5)
        rrows = np.zeros((NPAIR, S, H), dtype=FP8E5)
        for p, (i, j) in enumerate(PAIRS):
            lv, rv = lv_list[i], rv_list[j]
            ml = max(float(np.abs(lv).max()), 1e-30)
            mr = max(float(np.abs(rv).max()), 1e-30)
            al = np.round((GS + np.log2(mr) - np.log2(ml)) / 2.0)
            al = min(al, np.floor(np.log2(57344.0 / ml)))
            al = max(al, GS - np.floor(np.log2(57344.0 / mr)))
            lrows[p] = (lv * f32(2.0 ** al)).astype(FP8E5)
            rrows[p] = (rv * f32(2.0 ** (GS - al))).astype(FP8E5)
        return lrows, rrows

    la, ra = pair_rows(upL, kmL)
    lb, rb_ = pair_rows(bpL, cL)
    r3l = np.stack([la, lb], axis=1)                     # [NPAIR, 2, S, H]
    r3r = np.stack([ra, rb_], axis=1)
    r3l = np.ascontiguousarray(r3l.transpose(3, 0, 1, 2))  # [H, NPAIR, 2, S]
    r3r = np.ascontiguousarray(r3r.transpose(3, 0, 1, 2))

    vs_full = np.repeat(vs, VG, axis=0)
    vm_full = np.repeat(vm, VG, axis=0)
    vd = v * vs_full + vm_full            # f32 [S, H, D]
    vdt = vd.transpose(1, 0, 2).reshape(H, NKT, 128, D)
    vdt = np.ascontiguousarray(vdt.transpose(0, 2, 1, 3)).astype(BF16)

    mask = np.triu(np.full((128, 128), -1e30, dtype=f32), k=1)
    ident = np.eye(128, dtype=np.float32).astype(BF16)

    return dict(qT=qT, kTh=kTh, qp8=qp8, kl8=kl8, rho=rho_r, nrho=nrho_r,
                r2l=r2l, r2r=r2r, r3l=r3l, r3r=r3r, vv=vdt, mask=mask,
                neg128=np.full((128, 1), -128.0, dtype=f32),
                ident=ident, vd_f32=vd)


def _host_last_row(query, key, qmin, qscale, kmin, kscale, vd_f32):
    """Exact reference math (numpy f32) for the single non-causal row."""
    f32 = np.float32
    i = S - 1
    out = np.zeros((H, D), dtype=f32)
    for h in range(H):
        qd = query[i, 0, h, :].astype(f32) * f32(qscale[i, 0, h]) + f32(qmin[i, 0, h])
        kd = key[:, 0, h, :].astype(f32) * kscale[:, 0, h].astype(f32)[:, None] \
            + kmin[:, 0, h].astype(f32)[:, None]
        s = (kd @ qd).astype(f32) * f32(1.0 / math.sqrt(D))
        e = np.exp(s - s.max(), dtype=f32)
        p = (e / e.sum(dtype=f32)).astype(f32)
        pmax, pmin_ = p.max(), p.min()
        pscale = (pmax - pmin_) / f32(P_LEVELS)
        safe = pscale if pscale > 0 else f32(1.0)
        pq = np.floor((p - pmin_) / safe).astype(f32)
        pd = pq * pscale + pmin_
        out[h] = pd @ vd_f32[:, h, :]
    return out


def _reference_numpy(query, key, value, qmin, qscale, kmin, kscale,
                     vmin, vscale, causal):
    f32 = np.float32
    q = query[:, 0, :, :].astype(f32)
    k = key[:, 0, :, :].astype(f32)
    v = value[:, 0, :, :].astype(f32)
    out = np.zeros((S, B, H * D), dtype=f32)
    vs_full = np.repeat(vscale[:, 0, :, :].astype(f32), VG, axis=0)
    vm_full = np.repeat(vmin[:, 0, :, :].astype(f32), VG, axis=0)
    for h in range(H):
        qd = q[:, h, :] * qscale[:, 0, h].astype(f32)[:, None] + qmin[:, 0, h].astype(f32)[:, None]
        kd = k[:, h, :] * kscale[:, 0, h].astype(f32)[:, None] + kmin[:, 0, h].astype(f32)[:, None]
        s = (qd @ kd.T) * f32(1.0 / math.sqrt(D))
        if causal:
            s = np.where(np.tril(np.ones((S, S), dtype=bool)), s, f32(-1e30))
        e = np.exp(s - s.max(axis=1, keepdims=True), dtype=f32)
        p = e / e.sum(axis=1, keepdims=True, dtype=f32)
        pmax = p.max(axis=1, keepdims=True)
        pmin_ = p.min(axis=1, keepdims=True)
        pscale = (pmax - pmin_) / f32(P_LEVELS)
        safe = np.where(pscale > 0, pscale, f32(1.0))
        pd = np.floor((p - pmin_) / safe) * pscale + pmin_
        vd = v[:, h, :] * vs_full[:, h, :] + vm_full[:, h, :]
        out[:, 0, h * D:(h + 1) * D] = pd.astype(f32) @ vd
    return out


def kernel(query, key, value, qmin, qscale, kmin, kscale, vmin, vscale,
           causal):
    global _COMPILED
    causal_i = int(np.asarray(causal))
    if causal_i != 1:
        return _reference_numpy(query, key, value, qmin, qscale, kmin,
                                kscale, vmin, vscale, causal_i)

    prep = _host_prep(query, key, value, qmin, qscale, kmin, kscale,
                      vmin, vscale)

    if _COMPILED is None:
        _COMPILED = _build_graph()
    nc = _COMPILED

    in_maps = []
    for core in range(N_CORES):
        hs = slice(core * HPC, (core + 1) * HPC)
        in_maps.append({
            "qT": np.ascontiguousarray(prep["qT"][hs]),
            "kTh": np.ascontiguousarray(prep["kTh"][hs]),
            "qp8": np.ascontiguousarray(prep["qp8"][hs]),
            "kl8": np.ascontiguousarray(prep["kl8"][hs]),
            "rho": np.ascontiguousarray(prep["rho"][hs]),
            "nrho": np.ascontiguousarray(prep["nrho"][hs]),
            "r2l": np.ascontiguousarray(prep["r2l"][hs]),
            "r2r": np.ascontiguousarray(prep["r2r"][hs]),
            "r3l": np.ascontiguousarray(prep["r3l"][hs]),
            "r3r": np.ascontiguousarray(prep["r3r"][hs]),
            "vv": np.ascontiguousarray(prep["vv"][hs]),
            "mask": prep["mask"],
            "neg128": prep["neg128"],
            "ident": prep["ident"],
        })

    from concourse.bass_utils import run_bass_kernel_spmd
    trace = bool(int(os.environ.get("KERNEL_TRACE", "0")))
    res = run_bass_kernel_spmd(nc, in_maps, core_ids=list(range(N_CORES)),
                               trace=trace)
    if res.exec_time_ns is not None:
        kernel.last_exec_ns = res.exec_time_ns
        print(f"HW exec time: {res.exec_time_ns} ns")

    out = np.zeros((S, B, H * D), dtype=np.float32)
    for core in range(N_CORES):
        o = np.asarray(res.results[core]["out"], dtype=np.float32)
        for j in range(HPC):
            h = core * HPC + j
            out[:, 0, h * D:(h + 1) * D] = o[j].reshape(S, D)

    last = _host_last_row(query, key, qmin, qscale, kmin, kscale,
                          prep["vd_f32"])
    for h in range(H):
        out[S - 1, 0, h * D:(h + 1) * D] = last[h]
    return out


kernel.last_exec_ns = None


# revision 52
# speedup vs baseline: 1.3987x; 1.0370x over previous
"""
Sparse (quantized) attention on 8 Trainium2 NeuronCores.

Strategy: head-parallel sharding. 16 (b,h) heads -> 2 heads per core, no
collectives. Per head the device computes, for each 128-query row-block
(causal: only the first rb+1 key tiles):

  scores*2^18 in PSUM via
    P1: bf16-matmul(q*2^e, 2^18*bf16(ks*k))            (exact products)
    P2: fp8e4 DoubleRow matmul of the (q hi,lo) pair against the
        duplicated fp8 k-lo residual (2^18 split 2^8 * 2^10)
    P3: rank-6 bf16 correction (u/rho)*km + (b/rho)*c  (hi/lo split)
  row max m (DVE per-chunk reduce), nm = -rho'*m + ln16 + delta
  e16 = exp(rho'*sc + nm)  in [0, 16.003], accum_out -> z = 16*Z
  t = bf16(e16 + 127.5)    (Pool; bf16 round-to-nearest == floor+128)
  fT via PE transpose of t, eviction = relu(t^ - 128) (ACT/DVE split)
  PV = fT @ vd (bf16), out = PV * (1/z)

Exact in real arithmetic because for causal rows pmin=0, so
pd = floor(16*e)/(16*Z).  The single non-causal row (s=S-1) is computed
on the host. V dequant (v*vs+vm) is folded on the host into bf16 vd.
"""

import math
import os

import numpy as np
import ml_dtypes

S, B, H, D = 2048, 1, 16, 128
VG = 128
G = S // VG
P_LEVELS = 16.0
N_CORES = 8
HPC = H // N_CORES  # heads per core = 2
RB = 128            # row-block (query tile) size
NRB = S // RB       # 16 row-blocks
NKT = S // 128      # 16 key tiles

BF16 = ml_dtypes.bfloat16
FP8E4 = ml_dtypes.float8_e4m3
FP8E5 = ml_dtypes.float8_e5m2
LMAX = 6            # corr fp8e5 level pairs (i+j <= LMAX)
NLV = LMAX + 1
PAIRS = [(i, j) for i in range(NLV) for j in range(NLV) if i + j <= LMAX]
NPAIR = len(PAIRS)  # 28 cells, 2 products per cell
DELTA = 2e-4
LN16D = float(np.log(np.float64(16.0)) + DELTA)
GS = 18             # global log2 scale on the scores PSUM
QS8 = 8             # q-pair fp8 pre-scale (2^QS8)
KS8 = GS - QS8      # k-lo fp8 pre-scale

_COMPILED = None


def _build_graph():
    import concourse.bass as bass
    import concourse.bacc as bacc
    import concourse.tile as tile
    import concourse.mybir as mybir

    f32 = mybir.dt.float32
    bf16 = mybir.dt.bfloat16
    fp8e4 = mybir.dt.float8e4
    fp8e5 = mybir.dt.float8e5
    Alu = mybir.AluOpType
    Act = mybir.ActivationFunctionType

    nc = bacc.Bacc("TRN2", target_bir_lowering=False, debug=False,
                   num_devices=N_CORES)

    qT_d = nc.declare_dram_parameter("qT", [HPC, 128, S], bf16, isOutput=False)
    kTh_d = nc.declare_dram_parameter("kTh", [HPC, 128, S], bf16, isOutput=False)
    qp8_d = nc.declare_dram_parameter("qp8", [HPC, 128, 2, S], fp8e4,
                                      isOutput=False)
    kl8_d = nc.declare_dram_parameter("kl8", [HPC, 128, 2, S], fp8e4,
                                      isOutput=False)
    rho_d = nc.declare_dram_parameter("rho", [HPC, 128, NRB], f32, isOutput=False)
    nrho_d = nc.declare_dram_parameter("nrho", [HPC, 128, NRB], f32, isOutput=False)
    r2l_d = nc.declare_dram_parameter("r2l", [HPC, 6, S], bf16, isOutput=False)
    r2r_d = nc.declare_dram_parameter("r2r", [HPC, 6, S], bf16, isOutput=False)
    r3l_d = nc.declare_dram_parameter("r3l", [HPC, NPAIR, 2, S], fp8e5,
                                      isOutput=False)
    r3r_d = nc.declare_dram_parameter("r3r", [HPC, NPAIR, 2, S], fp8e5,
                                      isOutput=False)
    v_d = nc.declare_dram_parameter("vv", [HPC, 128, NKT, 128], bf16,
                                    isOutput=False)
    mask_d = nc.declare_dram_parameter("mask", [128, 128], f32, isOutput=False)
    n128_d = nc.declare_dram_parameter("neg128", [128, 1], f32, isOutput=False)
    id_d = nc.declare_dram_parameter("ident", [128, 128], bf16, isOutput=False)
    out_d = nc.declare_dram_parameter("out", [HPC, NRB, 128, 128], f32,
                                      isOutput=True)

    CHUNK = int(os.environ.get("K_CHUNK", 1024))
    TRMODE = os.environ.get("K_TR", "pe")        # dma | pe transpose path
    PS_S = int(os.environ.get("K_PSS", 3))
    PS_T = int(os.environ.get("K_PST", 1 if TRMODE == "pe" else 0))
    PS_V = int(os.environ.get("K_PSV", 1 if TRMODE == "pe" else 2))
    WB = int(os.environ.get("K_WB", 7))
    TG = int(os.environ.get("K_TG", 8))          # tiles per transpose group
    EV_MOD = int(os.environ.get("K_EVM", 4))     # eviction: gi%EV_MOD==EV_ACT -> ACT
    EV_ACT = int(os.environ.get("K_EVA", 1))
    MG_ENG = os.environ.get("K_MG", "g")         # magic-add engine: g=Pool
    RL_ENG = os.environ.get("K_RL", "v")         # relu engine (dma path)
    USE_FP8_P2 = int(os.environ.get("K_FP8P2", 1))
    USE_FP8_P3 = int(os.environ.get("K_FP8P3", 1))

    import contextlib
    with tile.TileContext(nc) as tc:
        with contextlib.ExitStack() as es:
            constp = es.enter_context(tc.tile_pool(name="const", bufs=1))
            headp = es.enter_context(tc.tile_pool(name="heads", bufs=2))
            workp = es.enter_context(tc.tile_pool(name="work", bufs=WB))
            statp = es.enter_context(
                tc.tile_pool(name="stat", bufs=int(os.environ.get("K_SB", 8))))
            ps_s = es.enter_context(
                tc.tile_pool(name="ps_s", bufs=PS_S, space="PSUM"))
            ps_v = es.enter_context(
                tc.tile_pool(name="ps_v", bufs=PS_V, space="PSUM"))
            ps_t = (es.enter_context(
                tc.tile_pool(name="ps_t", bufs=PS_T, space="PSUM"))
                if PS_T > 0 else None)
            mask_sb = constp.tile([128, 128], f32, tag="mask")
            nc.sync.dma_start(mask_sb[:], mask_d[:])
            n128_sb = constp.tile([128, 1], f32, tag="neg128")
            nc.sync.dma_start(n128_sb[:], n128_d[:])
            id_sb = constp.tile([128, 128], bf16, tag="ident")
            nc.sync.dma_start(id_sb[:], id_d[:])
            # warm the ACT exp table so LoadActFuncSet is off the critical path
            warm = constp.tile([128, 1], f32, tag="warm")
            nc.gpsimd.memset(warm[:], 0.0)
            nc.scalar.activation(warm[:], warm[:], Act.Exp)

            gi = 0  # global transpose-group counter (eviction engine split)
            SPL = int(os.environ.get("K_SPL", 896))  # first-piece columns
            ILV = int(os.environ.get("K_ILV", 0))    # interleave the 2 heads
            hdat = []
            for h in range(HPC):
                d = {}
                d["qT"] = headp.tile([128, S], bf16, tag="qT", name=f"qT{h}")
                d["kTh"] = headp.tile([128, S], bf16, tag="kTh", name=f"kTh{h}")
                if USE_FP8_P2:
                    d["qp8"] = headp.tile([128, 2, S], fp8e4, tag="qp8", name=f"qp8{h}")
                    d["kl8"] = headp.tile([128, 2, S], fp8e4, tag="kl8", name=f"kl8{h}")
                else:
                    d["kTl"] = headp.tile([128, S], bf16, tag="kTl", name=f"kTl{h}")
                d["rho"] = headp.tile([128, NRB], f32, tag="rho", name=f"rho{h}")
                d["nrho"] = headp.tile([128, NRB], f32, tag="nrho", name=f"nrho{h}")
                if USE_FP8_P3:
                    d["r2l"] = headp.tile([NPAIR, 2, S], fp8e5, tag="r2l",
                                          name=f"r3l{h}")
                    d["r2r"] = headp.tile([NPAIR, 2, S], fp8e5, tag="r2r",
                                          name=f"r3r{h}")
                else:
                    d["r2l"] = headp.tile([6, S], bf16, tag="r2l", name=f"r2l{h}")
                    d["r2r"] = headp.tile([6, S], bf16, tag="r2r", name=f"r2r{h}")
                d["v"] = headp.tile([128, NKT, 128], bf16, tag="vv", name=f"vv{h}")
                hdat.append(d)
            # stage the first SPL columns of the score operands (both heads)
            # so the PE can start while the bulk still streams in
            for h in range(HPC):
                spl = SPL if (h == 0 or ILV) and SPL > 0 else 0
                d = hdat[h]
                if spl:
                    nc.sync.dma_start(d["qT"][:, :spl], qT_d[h][:, :spl])
                    nc.sync.dma_start(d["kTh"][:, :spl], kTh_d[h][:, :spl])
                    if USE_FP8_P2:
                        nc.sync.dma_start(d["qp8"][:, :, :spl],
                                          qp8_d[h][:, :, :spl])
                        nc.sync.dma_start(d["kl8"][:, :, :spl],
                                          kl8_d[h][:, :, :spl])
                    if USE_FP8_P3:
                        nc.sync.dma_start(d["r2l"][:, :, :spl],
                                          r3l_d[h][:, :, :spl])
                        nc.sync.dma_start(d["r2r"][:, :, :spl],
                                          r3r_d[h][:, :, :spl])
                    else:
                        nc.sync.dma_start(d["r2l"][:, :spl], r2l_d[h][:, :spl])
                        nc.sync.dma_start(d["r2r"][:, :spl], r2r_d[h][:, :spl])
            for h in range(HPC):
                spl = SPL if (h == 0 or ILV) and SPL > 0 else 0
                d = hdat[h]
                nc.sync.dma_start(d["rho"][:], rho_d[h])
                nc.sync.dma_start(d["nrho"][:], nrho_d[h])
                if spl:
                    nc.sync.dma_start(d["qT"][:, spl:], qT_d[h][:, spl:])
                    nc.sync.dma_start(d["kTh"][:, spl:], kTh_d[h][:, spl:])
                    if USE_FP8_P2:
                        nc.sync.dma_start(d["qp8"][:, :, spl:],
                                          qp8_d[h][:, :, spl:])
                        nc.sync.dma_start(d["kl8"][:, :, spl:],
                                          kl8_d[h][:, :, spl:])
                    if USE_FP8_P3:
                        nc.sync.dma_start(d["r2l"][:, :, spl:],
                                          r3l_d[h][:, :, spl:])
                        nc.sync.dma_start(d["r2r"][:, :, spl:],
                                          r3r_d[h][:, :, spl:])
                    else:
                        nc.sync.dma_start(d["r2l"][:, spl:], r2l_d[h][:, spl:])
                        nc.sync.dma_start(d["r2r"][:, spl:], r2r_d[h][:, spl:])
                else:
                    nc.sync.dma_start(d["qT"][:], qT_d[h])
                    nc.sync.dma_start(d["kTh"][:], kTh_d[h])
                    if USE_FP8_P2:
                        nc.sync.dma_start(d["qp8"][:], qp8_d[h])
                        nc.sync.dma_start(d["kl8"][:], kl8_d[h])
                    if USE_FP8_P3:
                        nc.sync.dma_start(d["r2l"][:], r3l_d[h])
                        nc.sync.dma_start(d["r2r"][:], r3r_d[h])
                    else:
                        nc.sync.dma_start(d["r2l"][:], r2l_d[h])
                        nc.sync.dma_start(d["r2r"][:], r2r_d[h])
                if not USE_FP8_P2:
                    nc.sync.dma_start(d["kTl"][:], qp8_d[h])  # unused path
                nc.sync.dma_start(d["v"][:], v_d[h])

            if ILV:
                order = [(it % HPC, it // HPC) for it in range(HPC * NRB)]
            else:
                order = [(h, rb) for h in range(HPC) for rb in range(NRB)]
            if int(os.environ.get("K_REV", 0)):
                order = [(h, NRB - 1 - rb if h % 2 else rb)
                         for (h, rb) in order]
            if int(os.environ.get("K_DESC", 0)):
                order = [(h, NRB - 1 - rb) for (h, rb) in order]

            def stage1(h, rb):
                    d = hdat[h]
                    qT_sb = d["qT"]
                    kTh_sb = d["kTh"]
                    if USE_FP8_P2:
                        qp8_sb = d["qp8"]
                        kl8_sb = d["kl8"]
                    else:
                        kTl_sb = d["kTl"]
                    rho_sb = d["rho"]
                    nrho_sb = d["nrho"]
                    r2l_sb = d["r2l"]
                    r2r_sb = d["r2r"]
                    T = rb + 1
                    NK = T * 128
                    q0 = rb * 128
                    nch = (NK + CHUNK - 1) // CHUNK

                    # Rescale groups: each non-diagonal chunk-rest plus the
                    # 128-col diagonal tile exponentiate against their own
                    # local max as soon as their matmuls (+fused mask/max for
                    # the diagonal, via tensor_tensor_reduce) complete; the
                    # per-group factor s1_g = exp(rho*m_g + nm) folds into
                    # the Pool magic multiply.  Frees PSUM banks early and
                    # takes the diagonal mask off the big chunk's chain.
                    mx = statp.tile([128, 6], f32, tag="mx")
                    nm = statp.tile([128, 1], f32, tag="nm")
                    e = workp.tile([128, S], f32, tag="e")
                    zc = statp.tile([128, 6], f32, tag="zc")
                    groups = []  # (sc_tile, k0 global, off in tile, kn, diag)
                    for c in range(nch):
                        k0 = c * CHUNK
                        kn = min(NK, k0 + CHUNK) - k0
                        sc = ps_s.tile([128, CHUNK], f32, tag="sc")
                        RSP = int(os.environ.get("K_RSP", 0))
                        split = RSP and kn > 512
                        for n0 in range(0, kn, 512):
                            if n0 == 512 and split:
                                # first-half max overlaps second-half matmuls
                                nc.vector.tensor_reduce(
                                    mx[:, 4:5], sc[:, :512],
                                    axis=mybir.AxisListType.X, op=Alu.max)
                            n1 = min(kn, n0 + 512)
                            nc.tensor.matmul(sc[:, n0:n1],
                                             qT_sb[:, q0:q0 + 128],
                                             kTh_sb[:, k0 + n0:k0 + n1],
                                             start=True, stop=False)
                            if USE_FP8_P2:
                                nc.tensor.matmul(
                                    sc[:, n0:n1],
                                    qp8_sb[:, :, q0:q0 + 128],
                                    kl8_sb[:, :, k0 + n0:k0 + n1],
                                    start=False, stop=False,
                                    perf_mode=mybir.MatmulPerfMode.DoubleRow)
                            else:
                                nc.tensor.matmul(sc[:, n0:n1],
                                                 qT_sb[:, q0:q0 + 128],
                                                 kTl_sb[:, k0 + n0:k0 + n1],
                                                 start=False, stop=False)
                            if USE_FP8_P3:
                                nc.tensor.matmul(
                                    sc[:, n0:n1],
                                    r2l_sb[:, :, q0:q0 + 128],
                                    r2r_sb[:, :, k0 + n0:k0 + n1],
                                    start=False, stop=True,
                                    perf_mode=mybir.MatmulPerfMode.DoubleRow)
                            else:
                                nc.tensor.matmul(sc[:, n0:n1],
                                                 r2l_sb[:, q0:q0 + 128],
                                                 r2r_sb[:, k0 + n0:k0 + n1],
                                                 start=False, stop=True)
                        if c == nch - 1:
                            nc.vector.tensor_add(sc[:, kn - 128:kn],
                                                 sc[:, kn - 128:kn],
                                                 mask_sb[:])
                        if split:
                            nc.vector.tensor_reduce(
                                mx[:, 5:6], sc[:, 512:kn],
                                axis=mybir.AxisListType.X, op=Alu.max)
                            nc.vector.tensor_reduce(
                                mx[:, c:c + 1], mx[:, 4:6],
                                axis=mybir.AxisListType.X, op=Alu.max)
                        else:
                            nc.vector.tensor_reduce(mx[:, c:c + 1],
                                                    sc[:, :kn],
                                                    axis=mybir.AxisListType.X,
                                                    op=Alu.max)
                        groups.append((sc, k0, 0, kn, False))
                        if nch > 1:
                            nmc = statp.tile([128, 1], f32, tag="nmc")
                            nc.vector.tensor_scalar(nmc[:], mx[:, c:c + 1],
                                                    nrho_sb[:, rb:rb + 1],
                                                    None, Alu.mult)
                            bias_ap = nmc[:]
                        else:
                            nc.vector.tensor_scalar(nm[:], mx[:, 0:1],
                                                    nrho_sb[:, rb:rb + 1],
                                                    LN16D,
                                                    Alu.mult, Alu.add)
                            bias_ap = nm[:]
                        nc.scalar.activation(e[:, k0:k0 + kn],
                                             sc[:, :kn],
                                             Act.Exp,
                                             bias=bias_ap,
                                             scale=rho_sb[:, rb:rb + 1],
                                             accum_out=zc[:, c:c + 1])
                    G = len(groups)
                    if G > 1:
                        m = statp.tile([128, 1], f32, tag="m")
                        nc.vector.tensor_reduce(m[:], mx[:, :G],
                                                axis=mybir.AxisListType.X,
                                                op=Alu.max)
                        nc.vector.tensor_scalar(nm[:], m[:],
                                                nrho_sb[:, rb:rb + 1],
                                                LN16D,
                                                Alu.mult, Alu.add)

                    r = statp.tile([128, 1], f32, tag="r")
                    t = workp.tile([128, S], bf16, tag="t")
                    eng = nc.gpsimd if MG_ENG == "g" else nc.vector
                    MGW = int(os.environ.get("K_MGW", 640))  # magic op width
                    if G > 1:
                        # s1_g = exp(rho*m_g + nm); winner = exactly 16*e^d
                        s1 = statp.tile([128, 6], f32, tag="s1")
                        nc.scalar.activation(s1[:, :G], mx[:, :G],
                                             Act.Exp,
                                             bias=nm[:],
                                             scale=rho_sb[:, rb:rb + 1])
                        w = statp.tile([128, 6], f32, tag="w")
                        nc.vector.tensor_tensor(w[:, :G], zc[:, :G],
                                                s1[:, :G], op=Alu.mult)
                        z = statp.tile([128, 1], f32, tag="z")
                        nc.vector.tensor_reduce(z[:], w[:, :G],
                                                axis=mybir.AxisListType.X,
                                                op=Alu.add)
                        nc.vector.reciprocal(r[:], z[:])
                        for g, (sc, k0, off, kn, diag) in enumerate(groups):
                            for m0 in range(0, kn, MGW):
                                m1 = min(kn, m0 + MGW)
                                eng.tensor_scalar(t[:, k0 + m0:k0 + m1],
                                                  e[:, k0 + m0:k0 + m1],
                                                  s1[:, g:g + 1], 127.5,
                                                  Alu.mult, Alu.add)
                    else:
                        nc.vector.reciprocal(r[:], zc[:, 0:1])
                        for m0 in range(0, NK, MGW):
                            m1 = min(NK, m0 + MGW)
                            eng.tensor_scalar(t[:, m0:m1], e[:, m0:m1],
                                              127.5, None, Alu.add)
                    return dict(t=t, r=r, groups=groups)

            def stage2(h, rb, ctx):
                    nonlocal gi
                    d = hdat[h]
                    v_sb = d["v"]
                    t = ctx["t"]
                    r = ctx["r"]
                    groups = ctx["groups"]
                    T = rb + 1
                    NK = T * 128

                    fT = workp.tile([128, NKT, 128], bf16, tag="fT")
                    if TRMODE == "dma":
                        # f = relu(t - 128) in SBUF (DVE 4x bf16), then the
                        # DMA crossbar transposes all T tiles in one shot
                        f = workp.tile([128, S], bf16, tag="f")
                        for g, (sc, k0, off, kn, diag) in enumerate(groups):
                            reng = nc.vector if RL_ENG == "v" else nc.gpsimd
                            reng.tensor_scalar(f[:, k0:k0 + kn],
                                               t[:, k0:k0 + kn],
                                               128.0, 0.0,
                                               Alu.subtract, Alu.max)
                        nc.sync.dma_start_transpose(fT[:, :T, :], f[:, :NK])
                    else:
                        for t0 in range(0, T, TG):
                            tn = min(TG, T - t0)
                            ptr = ps_t.tile([128, TG * 128], bf16, tag="tr")
                            for i in range(tn):
                                tt = t0 + i
                                nc.tensor.transpose(
                                    ptr[:, i * 128:(i + 1) * 128],
                                    t[:, tt * 128:(tt + 1) * 128],
                                    id_sb[:])
                            dst = fT[:, t0:t0 + tn, :]
                            if gi % EV_MOD == EV_ACT:
                                nc.scalar.activation(dst, ptr[:, :tn * 128],
                                                     Act.Relu,
                                                     bias=n128_sb[:],
                                                     scale=1.0)
                            else:
                                nc.vector.tensor_scalar(dst,
                                                        ptr[:, :tn * 128],
                                                        128.0, 0.0,
                                                        Alu.subtract, Alu.max)
                            gi += 1

                    pv = ps_v.tile([128, 128], f32, tag="pv")
                    for tt in range(T):
                        nc.tensor.matmul(pv[:],
                                         fT[:, tt, :],
                                         v_sb[:, tt, :],
                                         start=(tt == 0), stop=(tt == T - 1))

                    o = workp.tile([128, 128], f32, tag="o")
                    if os.environ.get("K_OSC", "a") == "a":
                        nc.scalar.mul(o[:], pv[:], r[:])
                    else:
                        nc.vector.tensor_scalar(o[:], pv[:], r[:], None,
                                                Alu.mult)
                    nc.sync.dma_start(out_d[h, rb], o[:])

            # software pipeline: emit stage1 of upcoming row-blocks before
            # stage2 of earlier ones so ready reduces/exps aren't queued
            # behind evictions that wait on the magic->transpose chain
            PIPE = int(os.environ.get("K_PIPE", 3))
            pend = []
            for (h, rb) in order:
                ctx = stage1(h, rb)
                pend.append((h, rb, ctx))
                if len(pend) >= PIPE:
                    h2, rb2, c2 = pend.pop(0)
                    stage2(h2, rb2, c2)
            for h2, rb2, c2 in pend:
                stage2(h2, rb2, c2)

    nc.compile()
    return nc


def _host_prep(query, key, value, qmin, qscale, kmin, kscale, vmin, vscale):
    """Builds per-head device inputs, stacked [H, ...]."""
    f32 = np.float32
    q = query[:, 0, :, :].astype(f32)     # [S, H, D]
    k = key[:, 0, :, :].astype(f32)
    v = value[:, 0, :, :].astype(f32)
    qs = qscale[:, 0, :].astype(f32)      # [S, H]
    qm = qmin[:, 0, :].astype(f32)
    ks = kscale[:, 0, :].astype(f32)
    km = kmin[:, 0, :].astype(f32)
    vs = vscale[:, 0, :, :].astype(f32)   # [G, H, D]
    vm = vmin[:, 0, :, :].astype(f32)

    rsd = f32(1.0 / math.sqrt(D))
    a = qs * rsd
    b = qm * rsd
    sq = q.sum(axis=2)
    sk = k.sum(axis=2)
    u = a * sq + b * f32(D)
    c = ks * sk

    # q side: a = rho * 2^e; q2e = q * 2^e exact in bf16.
    e_i = np.round(np.log2(a))
    two_e = np.exp2(e_i).astype(f32)
    rho = (a / two_e).astype(f32)
    q2e = q * two_e[:, :, None]                         # [S, H, D] exact
    qT = np.ascontiguousarray(q2e.transpose(1, 2, 0)).astype(BF16)  # [H,D,S]

    # scores PSUM is globally scaled by 2^GS; rho' = rho * 2^-GS
    rho_s = (rho * f32(2.0 ** -GS)).astype(f32)

    # k hi: bf16(ks*k) * 2^GS (exact exponent shift after rounding)
    kp = (k * ks[:, :, None]).astype(f32)
    kph = kp.astype(BF16).astype(f32)
    kTh = np.ascontiguousarray((kph * f32(2.0 ** GS)).transpose(1, 2, 0)
                               ).astype(BF16)           # [H, D, S]

    # k lo residual as fp8 pair path: q-pair (hi/lo nibbles of q2e * 2^QS8)
    q2e_s = (q2e * f32(2.0 ** QS8)).astype(f32)
    q1 = q2e_s.astype(FP8E4).astype(f32)
    q2 = (q2e_s - q1).astype(FP8E4)
    qp8 = np.stack([q1.astype(FP8E4), q2], axis=2)      # [S, H, 2, D]
    qp8 = np.ascontiguousarray(qp8.transpose(1, 3, 2, 0))  # [H, D, 2, S]

    kpl = ((kp - kph) * f32(2.0 ** KS8)).astype(FP8E4)  # [S, H, D]
    kl8 = np.stack([kpl, kpl], axis=2)                  # [S, H, 2, D]
    kl8 = np.ascontiguousarray(kl8.transpose(1, 3, 2, 0))  # [H, D, 2, S]

    rho_r = np.ascontiguousarray(
        rho_s.T.reshape(H, NRB, 128).transpose(0, 2, 1)).astype(f32)
    nrho_r = np.ascontiguousarray(-rho_r)

    def hilo(x):
        xh = x.astype(BF16).astype(f32)
        xl = (x - xh).astype(BF16).astype(f32)
        return xh, xl

    # rank-6 correction, scaled 2^GS split as 2^9 per side
    s9 = f32(2.0 ** 9)
    up = (u / rho).astype(f32)
    bp = (b / rho).astype(f32)
    uh, ul = hilo(up * s9)
    bh, bl = hilo(bp * s9)
    kmh, kml = hilo(km * s9)
    ch, cl = hilo(c * s9)
    r2l = np.stack([uh, uh, ul, bh, bh, bl], axis=0)     # [6, S, H]
    r2r = np.stack([kmh, kml, kmh, ch, cl, ch], axis=0)
    r2l = np.ascontiguousarray(r2l.transpose(2, 0, 1)).astype(BF16)  # [H,6,S]
    r2r = np.ascontiguousarray(r2r.transpose(2, 0, 1)).astype(BF16)

    # corr as fp8e5 DoubleRow level pairs: corr*2^GS = sum over PAIRS of
    # (upL_i*2^a)(kmL_j*2^(GS-a)) + (bpL_i*2^a)(cL_j*2^(GS-a))
    def e5_levels(x):
        parts = []
        rr = x.astype(f32).copy()
        for _ in range(NLV):
            mmx = max(float(np.abs(rr).max()), 1e-30)
            sh = f32(2.0 ** np.floor(np.log2(28672.0 / mmx)))
            p = (rr * sh).astype(FP8E5).astype(f32) / sh
            parts.append(p)
            rr = rr - p
        return parts

    upL = e5_levels(up)
    bpL = e5_levels(bp)
    kmL = e5_levels(km)
    cL = e5_levels(c)

    def pair_rows(lv_list, rv_list):
        lrows = np.zeros((NPAIR, S, H), dtype=FP8E5)
        rrows = np.zeros((NPAIR, S, H), dtype=FP8E5)
        for p, (i, j) in enumerate(PAIRS):
            lv, rv = lv_list[i], rv_list[j]
            ml = max(float(np.abs(lv).max()), 1e-30)
            mr = max(float(np.abs(rv).max()), 1e-30)
            al = np.round((GS + np.log2(mr) - np.log2(ml)) / 2.0)
            al = min(al, np.floor(np.log2(57344.0 / ml)))
            al = max(al, GS - np.floor(np.log2(57344.0 / mr)))
            lrows[p] = (lv * f32(2.0 ** al)).astype(FP8E5)
            rrows[p] = (rv * f32(2.0 ** (GS - al))).astype(FP8E5)
        return lrows, rrows

    la, ra = pair_rows(upL, kmL)
    lb, rb_ = pair_rows(bpL, cL)
    r3l = np.stack([la, lb], axis=1)                     # [NPAIR, 2, S, H]
    r3r = np.stack([ra, rb_], axis=1)
    r3l = np.ascontiguousarray(r3l.transpose(3, 0, 1, 2))  # [H, NPAIR, 2, S]
    r3r = np.ascontiguousarray(r3r.transpose(3, 0, 1, 2))

    vs_full = np.repeat(vs, VG, axis=0)
    vm_full = np.repeat(vm, VG, axis=0)
    vd = v * vs_full + vm_full            # f32 [S, H, D]
    vdt = vd.transpose(1, 0, 2).reshape(H, NKT, 128, D)
    vdt = np.ascontiguousarray(vdt.transpose(0, 2, 1, 3)).astype(BF16)

    mask = np.triu(np.full((128, 128), -1e30, dtype=f32), k=1)
    ident = np.eye(128, dtype=np.float32).astype(BF16)

    return dict(qT=qT, kTh=kTh, qp8=qp8, kl8=kl8, rho=rho_r, nrho=nrho_r,
                r2l=r2l, r2r=r2r, r3l=r3l, r3r=r3r, vv=vdt, mask=mask,
                neg128=np.full((128, 1), -128.0, dtype=f32),
                ident=ident, vd_f32=vd)


def _host_last_row(query, key, qmin, qscale, kmin, kscale, vd_f32):
    """Exact reference math (numpy f32) for the single non-causal row."""
    f32 = np.float32
    i = S - 1
    out = np.zeros((H, D), dtype=f32)
    for h in range(H):
        qd = query[i, 0, h, :].astype(f32) * f32(qscale[i, 0, h]) + f32(qmin[i, 0, h])
        kd = key[:, 0, h, :].astype(f32) * kscale[:, 0, h].astype(f32)[:, None] \
            + kmin[:, 0, h].astype(f32)[:, None]
        s = (kd @ qd).astype(f32) * f32(1.0 / math.sqrt(D))
        e = np.exp(s - s.max(), dtype=f32)
        p = (e / e.sum(dtype=f32)).astype(f32)
        pmax, pmin_ = p.max(), p.min()
        pscale = (pmax - pmin_) / f32(P_LEVELS)
        safe = pscale if pscale > 0 else f32(1.0)
        pq = np.floor((p - pmin_) / safe).astype(f32)
        pd = pq * pscale + pmin_
        out[h] = pd @ vd_f32[:, h, :]
    return out


def _reference_numpy(query, key, value, qmin, qscale, kmin, kscale,
                     vmin, vscale, causal):
    f32 = np.float32
    q = query[:, 0, :, :].astype(f32)
    k = key[:, 0, :, :].astype(f32)
    v = value[:, 0, :, :].astype(f32)
    out = np.zeros((S, B, H * D), dtype=f32)
    vs_full = np.repeat(vscale[:, 0, :, :].astype(f32), VG, axis=0)
    vm_full = np.repeat(vmin[:, 0, :, :].astype(f32), VG, axis=0)
    for h in range(H):
        qd = q[:, h, :] * qscale[:, 0, h].astype(f32)[:, None] + qmin[:, 0, h].astype(f32)[:, None]
        kd = k[:, h, :] * kscale[:, 0, h].astype(f32)[:, None] + kmin[:, 0, h].astype(f32)[:, None]
        s = (qd @ kd.T) * f32(1.0 / math.sqrt(D))
        if causal:
            s = np.where(np.tril(np.ones((S, S), dtype=bool)), s, f32(-1e30))
        e = np.exp(s - s.max(axis=1, keepdims=True), dtype=f32)
        p = e / e.sum(axis=1, keepdims=True, dtype=f32)
        pmax = p.max(axis=1, keepdims=True)
        pmin_ = p.min(axis=1, keepdims=True)
        pscale = (pmax - pmin_) / f32(P_LEVELS)
        safe = np.where(pscale > 0, pscale, f32(1.0))
        pd = np.floor((p - pmin_) / safe) * pscale + pmin_
        vd = v[:, h, :] * vs_full[:, h, :] + vm_full[:, h, :]
        out[:, 0, h * D:(h + 1) * D] = pd.astype(f32) @ vd
    return out


def kernel(query, key, value, qmin, qscale, kmin, kscale, vmin, vscale,
           causal):
    global _COMPILED
    causal_i = int(np.asarray(causal))
    if causal_i != 1:
        return _reference_numpy(query, key, value, qmin, qscale, kmin,
                                kscale, vmin, vscale, causal_i)

    prep = _host_prep(query, key, value, qmin, qscale, kmin, kscale,
                      vmin, vscale)

    if _COMPILED is None:
        _COMPILED = _build_graph()
    nc = _COMPILED

    in_maps = []
    for core in range(N_CORES):
        hs = slice(core * HPC, (core + 1) * HPC)
        in_maps.append({
            "qT": np.ascontiguousarray(prep["qT"][hs]),
            "kTh": np.ascontiguousarray(prep["kTh"][hs]),
            "qp8": np.ascontiguousarray(prep["qp8"][hs]),
            "kl8": np.ascontiguousarray(prep["kl8"][hs]),
            "rho": np.ascontiguousarray(prep["rho"][hs]),
            "nrho": np.ascontiguousarray(prep["nrho"][hs]),
            "r2l": np.ascontiguousarray(prep["r2l"][hs]),
            "r2r": np.ascontiguousarray(prep["r2r"][hs]),
            "r3l": np.ascontiguousarray(prep["r3l"][hs]),
            "r3r": np.ascontiguousarray(prep["r3r"][hs]),
            "vv": np.ascontiguousarray(prep["vv"][hs]),
            "mask": prep["mask"],
            "neg128": prep["neg128"],
            "ident": prep["ident"],
        })

    from concourse.bass_utils import run_bass_kernel_spmd
    trace = bool(int(os.environ.get("KERNEL_TRACE", "0")))
    res = run_bass_kernel_spmd(nc, in_maps, core_ids=list(range(N_CORES)),
                               trace=trace)
    if res.exec_time_ns is not None:
        kernel.last_exec_ns = res.exec_time_ns
        print(f"HW exec time: {res.exec_time_ns} ns")

    out = np.zeros((S, B, H * D), dtype=np.float32)
    for core in range(N_CORES):
        o = np.asarray(res.results[core]["out"], dtype=np.float32)
        for j in range(HPC):
            h = core * HPC + j
            out[:, 0, h * D:(h + 1) * D] = o[j].reshape(S, D)

    last = _host_last_row(query, key, qmin, qscale, kmin, kscale,
                          prep["vd_f32"])
    for h in range(H):
        out[S - 1, 0, h * D:(h + 1) * D] = last[h]
    return out


kernel.last_exec_ns = None


# revision 57
# speedup vs baseline: 1.4037x; 1.0036x over previous
"""
Sparse (quantized) attention on 8 Trainium2 NeuronCores.

Strategy: head-parallel sharding. 16 (b,h) heads -> 2 heads per core, no
collectives. Per head the device computes, for each 128-query row-block
(causal: only the first rb+1 key tiles), in 1024-col PSUM chunks:

  scores*2^18 in PSUM via
    P1: bf16-matmul(q*2^e, 2^18*bf16(ks*k))            (exact products)
    P2: fp8e4 DoubleRow matmul of the (q hi,lo nibble) pair against the
        duplicated fp8 k-lo residual (2^18 split as 2^8 * 2^10)
    P3: fp8e5 DoubleRow rank-56 correction (u/rho)*km + (b/rho)*c,
        7 e5m2 levels per factor, all level pairs with i+j <= 6
  per-chunk row max m_c (DVE reduce after the causal mask add),
  per-chunk e~ = exp(rho'*sc - rho'*m_c) (ACT, accum_out -> chunk sums),
  s1_c = exp(rho'*m_c + nm) with nm = -rho'*m + ln16 + delta rescales
  each chunk to the global max in the magic multiply:
  t = bf16(e~*s1_c + 127.5)  (Pool; bf16 round-to-nearest in [128,256)
      == floor+128), z = sum_c zc_c*s1_c, r = 1/z
  fT via PE transpose of t, eviction = relu(t^-128) (ACT/DVE split)
  PV = fT @ vd (bf16), out = PV * r

Exact in real arithmetic because for causal rows pmin=0, so
pd = floor(16*e)/(16*Z).  The single non-causal row (s=S-1) is computed
on the host. V dequant (v*vs+vm) is folded on the host into bf16 vd.
The two stages (scores+softmax+magic | transpose+PV+out) are software-
pipelined across row-blocks; head-0 operand DMAs are split so the PE
starts before the bulk finishes streaming.
"""

import math
import os

import numpy as np
import ml_dtypes

S, B, H, D = 2048, 1, 16, 128
VG = 128
G = S // VG
P_LEVELS = 16.0
N_CORES = 8
HPC = H // N_CORES  # heads per core = 2
RB = 128            # row-block (query tile) size
NRB = S // RB       # 16 row-blocks
NKT = S // 128      # 16 key tiles

BF16 = ml_dtypes.bfloat16
FP8E4 = ml_dtypes.float8_e4m3
FP8E5 = ml_dtypes.float8_e5m2
LMAX = 6            # corr fp8e5 level pairs (i+j <= LMAX)
NLV = LMAX + 1
PAIRS = [(i, j) for i in range(NLV) for j in range(NLV) if i + j <= LMAX]
NPAIR = len(PAIRS)  # 28 cells, 2 products per cell
DELTA = 2e-4
LN16D = float(np.log(np.float64(16.0)) + DELTA)
GS = 18             # global log2 scale on the scores PSUM
QS8 = 8             # q-pair fp8 pre-scale (2^QS8)
KS8 = GS - QS8      # k-lo fp8 pre-scale

_COMPILED = None


def _build_graph():
    import concourse.bass as bass
    import concourse.bacc as bacc
    import concourse.tile as tile
    import concourse.mybir as mybir

    f32 = mybir.dt.float32
    bf16 = mybir.dt.bfloat16
    fp8e4 = mybir.dt.float8e4
    fp8e5 = mybir.dt.float8e5
    Alu = mybir.AluOpType
    Act = mybir.ActivationFunctionType

    nc = bacc.Bacc("TRN2", target_bir_lowering=False, debug=False,
                   num_devices=N_CORES)

    qT_d = nc.declare_dram_parameter("qT", [HPC, 128, S], bf16, isOutput=False)
    kTh_d = nc.declare_dram_parameter("kTh", [HPC, 128, S], bf16, isOutput=False)
    qp8_d = nc.declare_dram_parameter("qp8", [HPC, 128, 2, S], fp8e4,
                                      isOutput=False)
    kl8_d = nc.declare_dram_parameter("kl8", [HPC, 128, 2, S], fp8e4,
                                      isOutput=False)
    rho_d = nc.declare_dram_parameter("rho", [HPC, 128, NRB], f32, isOutput=False)
    nrho_d = nc.declare_dram_parameter("nrho", [HPC, 128, NRB], f32, isOutput=False)
    r2l_d = nc.declare_dram_parameter("r2l", [HPC, 6, S], bf16, isOutput=False)
    r2r_d = nc.declare_dram_parameter("r2r", [HPC, 6, S], bf16, isOutput=False)
    r3l_d = nc.declare_dram_parameter("r3l", [HPC, NPAIR, 2, S], fp8e5,
                                      isOutput=False)
    r3r_d = nc.declare_dram_parameter("r3r", [HPC, NPAIR, 2, S], fp8e5,
                                      isOutput=False)
    v_d = nc.declare_dram_parameter("vv", [HPC, 128, NKT, 128], bf16,
                                    isOutput=False)
    mask_d = nc.declare_dram_parameter("mask", [128, 128], f32, isOutput=False)
    n128_d = nc.declare_dram_parameter("neg128", [128, 1], f32, isOutput=False)
    id_d = nc.declare_dram_parameter("ident", [128, 128], bf16, isOutput=False)
    out_d = nc.declare_dram_parameter("out", [HPC, NRB, 128, 128], f32,
                                      isOutput=True)

    CHUNK = int(os.environ.get("K_CHUNK", 1024))
    TRMODE = os.environ.get("K_TR", "pe")        # dma | pe transpose path
    PS_S = int(os.environ.get("K_PSS", 3))
    PS_T = int(os.environ.get("K_PST", 1 if TRMODE == "pe" else 0))
    PS_V = int(os.environ.get("K_PSV", 1 if TRMODE == "pe" else 2))
    WB = int(os.environ.get("K_WB", 7))
    TG = int(os.environ.get("K_TG", 8))          # tiles per transpose group
    EV_MOD = int(os.environ.get("K_EVM", 4))     # eviction: gi%EV_MOD==EV_ACT -> ACT
    EV_ACT = int(os.environ.get("K_EVA", 1))
    MG_ENG = os.environ.get("K_MG", "g")         # magic-add engine: g=Pool
    RL_ENG = os.environ.get("K_RL", "v")         # relu engine (dma path)
    USE_FP8_P2 = int(os.environ.get("K_FP8P2", 1))
    USE_FP8_P3 = int(os.environ.get("K_FP8P3", 1))

    import contextlib
    with tile.TileContext(nc) as tc:
        with contextlib.ExitStack() as es:
            constp = es.enter_context(tc.tile_pool(name="const", bufs=1))
            headp = es.enter_context(tc.tile_pool(name="heads", bufs=2))
            workp = es.enter_context(tc.tile_pool(name="work", bufs=WB))
            statp = es.enter_context(
                tc.tile_pool(name="stat", bufs=int(os.environ.get("K_SB", 8))))
            ps_s = es.enter_context(
                tc.tile_pool(name="ps_s", bufs=PS_S, space="PSUM"))
            ps_v = es.enter_context(
                tc.tile_pool(name="ps_v", bufs=PS_V, space="PSUM"))
            ps_t = (es.enter_context(
                tc.tile_pool(name="ps_t", bufs=PS_T, space="PSUM"))
                if PS_T > 0 else None)
            mask_sb = constp.tile([128, 128], f32, tag="mask")
            nc.sync.dma_start(mask_sb[:], mask_d[:])
            n128_sb = constp.tile([128, 1], f32, tag="neg128")
            nc.sync.dma_start(n128_sb[:], n128_d[:])
            id_sb = constp.tile([128, 128], bf16, tag="ident")
            nc.sync.dma_start(id_sb[:], id_d[:])
            # warm the ACT exp table so LoadActFuncSet is off the critical path
            warm = constp.tile([128, 1], f32, tag="warm")
            nc.gpsimd.memset(warm[:], 0.0)
            nc.scalar.activation(warm[:], warm[:], Act.Exp)

            gi = 0  # global transpose-group counter (eviction engine split)
            SPL = int(os.environ.get("K_SPL", 896))  # first-piece columns
            ILV = int(os.environ.get("K_ILV", 0))    # interleave the 2 heads
            hdat = []
            for h in range(HPC):
                d = {}
                d["qT"] = headp.tile([128, S], bf16, tag="qT", name=f"qT{h}")
                d["kTh"] = headp.tile([128, S], bf16, tag="kTh", name=f"kTh{h}")
                if USE_FP8_P2:
                    d["qp8"] = headp.tile([128, 2, S], fp8e4, tag="qp8", name=f"qp8{h}")
                    d["kl8"] = headp.tile([128, 2, S], fp8e4, tag="kl8", name=f"kl8{h}")
                else:
                    d["kTl"] = headp.tile([128, S], bf16, tag="kTl", name=f"kTl{h}")
                d["rho"] = headp.tile([128, NRB], f32, tag="rho", name=f"rho{h}")
                d["nrho"] = headp.tile([128, NRB], f32, tag="nrho", name=f"nrho{h}")
                if USE_FP8_P3:
                    d["r2l"] = headp.tile([NPAIR, 2, S], fp8e5, tag="r2l",
                                          name=f"r3l{h}")
                    d["r2r"] = headp.tile([NPAIR, 2, S], fp8e5, tag="r2r",
                                          name=f"r3r{h}")
                else:
                    d["r2l"] = headp.tile([6, S], bf16, tag="r2l", name=f"r2l{h}")
                    d["r2r"] = headp.tile([6, S], bf16, tag="r2r", name=f"r2r{h}")
                d["v"] = headp.tile([128, NKT, 128], bf16, tag="vv", name=f"vv{h}")
                hdat.append(d)
            # stage the first SPL columns of the score operands (both heads)
            # so the PE can start while the bulk still streams in
            for h in range(HPC):
                spl = SPL if (h == 0 or ILV) and SPL > 0 else 0
                d = hdat[h]
                if spl:
                    nc.sync.dma_start(d["qT"][:, :spl], qT_d[h][:, :spl])
                    nc.sync.dma_start(d["kTh"][:, :spl], kTh_d[h][:, :spl])
                    if USE_FP8_P2:
                        nc.sync.dma_start(d["qp8"][:, :, :spl],
                                          qp8_d[h][:, :, :spl])
                        nc.sync.dma_start(d["kl8"][:, :, :spl],
                                          kl8_d[h][:, :, :spl])
                    if USE_FP8_P3:
                        nc.sync.dma_start(d["r2l"][:, :, :spl],
                                          r3l_d[h][:, :, :spl])
                        nc.sync.dma_start(d["r2r"][:, :, :spl],
                                          r3r_d[h][:, :, :spl])
                    else:
                        nc.sync.dma_start(d["r2l"][:, :spl], r2l_d[h][:, :spl])
                        nc.sync.dma_start(d["r2r"][:, :spl], r2r_d[h][:, :spl])
            for h in range(HPC):
                spl = SPL if (h == 0 or ILV) and SPL > 0 else 0
                d = hdat[h]
                nc.sync.dma_start(d["rho"][:], rho_d[h])
                nc.sync.dma_start(d["nrho"][:], nrho_d[h])
                if spl:
                    nc.sync.dma_start(d["qT"][:, spl:], qT_d[h][:, spl:])
                    nc.sync.dma_start(d["kTh"][:, spl:], kTh_d[h][:, spl:])
                    if USE_FP8_P2:
                        nc.sync.dma_start(d["qp8"][:, :, spl:],
                                          qp8_d[h][:, :, spl:])
                        nc.sync.dma_start(d["kl8"][:, :, spl:],
                                          kl8_d[h][:, :, spl:])
                    if USE_FP8_P3:
                        nc.sync.dma_start(d["r2l"][:, :, spl:],
                                          r3l_d[h][:, :, spl:])
                        nc.sync.dma_start(d["r2r"][:, :, spl:],
                                          r3r_d[h][:, :, spl:])
                    else:
                        nc.sync.dma_start(d["r2l"][:, spl:], r2l_d[h][:, spl:])
                        nc.sync.dma_start(d["r2r"][:, spl:], r2r_d[h][:, spl:])
                else:
                    nc.sync.dma_start(d["qT"][:], qT_d[h])
                    nc.sync.dma_start(d["kTh"][:], kTh_d[h])
                    if USE_FP8_P2:
                        nc.sync.dma_start(d["qp8"][:], qp8_d[h])
                        nc.sync.dma_start(d["kl8"][:], kl8_d[h])
                    if USE_FP8_P3:
                        nc.sync.dma_start(d["r2l"][:], r3l_d[h])
                        nc.sync.dma_start(d["r2r"][:], r3r_d[h])
                    else:
                        nc.sync.dma_start(d["r2l"][:], r2l_d[h])
                        nc.sync.dma_start(d["r2r"][:], r2r_d[h])
                if not USE_FP8_P2:
                    nc.sync.dma_start(d["kTl"][:], qp8_d[h])  # unused path
                nc.sync.dma_start(d["v"][:], v_d[h])

            if ILV:
                order = [(it % HPC, it // HPC) for it in range(HPC * NRB)]
            else:
                order = [(h, rb) for h in range(HPC) for rb in range(NRB)]
            if int(os.environ.get("K_REV", 0)):
                order = [(h, NRB - 1 - rb if h % 2 else rb)
                         for (h, rb) in order]
            if int(os.environ.get("K_DESC", 0)):
                order = [(h, NRB - 1 - rb) for (h, rb) in order]

            def stage1(h, rb):
                    d = hdat[h]
                    qT_sb = d["qT"]
                    kTh_sb = d["kTh"]
                    if USE_FP8_P2:
                        qp8_sb = d["qp8"]
                        kl8_sb = d["kl8"]
                    else:
                        kTl_sb = d["kTl"]
                    rho_sb = d["rho"]
                    nrho_sb = d["nrho"]
                    r2l_sb = d["r2l"]
                    r2r_sb = d["r2r"]
                    T = rb + 1
                    NK = T * 128
                    q0 = rb * 128
                    nch = (NK + CHUNK - 1) // CHUNK

                    # Rescale groups: each non-diagonal chunk-rest plus the
                    # 128-col diagonal tile exponentiate against their own
                    # local max as soon as their matmuls (+fused mask/max for
                    # the diagonal, via tensor_tensor_reduce) complete; the
                    # per-group factor s1_g = exp(rho*m_g + nm) folds into
                    # the Pool magic multiply.  Frees PSUM banks early and
                    # takes the diagonal mask off the big chunk's chain.
                    mx = statp.tile([128, 6], f32, tag="mx")
                    nm = statp.tile([128, 1], f32, tag="nm")
                    e = workp.tile([128, S], f32, tag="e")
                    zc = statp.tile([128, 6], f32, tag="zc")
                    groups = []  # (sc_tile, k0 global, off in tile, kn, diag)
                    for c in range(nch):
                        k0 = c * CHUNK
                        kn = min(NK, k0 + CHUNK) - k0
                        sc = ps_s.tile([128, CHUNK], f32, tag="sc")
                        RSP = int(os.environ.get("K_RSP", 0))
                        split = RSP and kn > 512
                        for n0 in range(0, kn, 512):
                            if n0 == 512 and split:
                                # first-half max overlaps second-half matmuls
                                nc.vector.tensor_reduce(
                                    mx[:, 4:5], sc[:, :512],
                                    axis=mybir.AxisListType.X, op=Alu.max)
                            n1 = min(kn, n0 + 512)
                            nc.tensor.matmul(sc[:, n0:n1],
                                             qT_sb[:, q0:q0 + 128],
                                             kTh_sb[:, k0 + n0:k0 + n1],
                                             start=True, stop=False)
                            if USE_FP8_P2:
                                nc.tensor.matmul(
                                    sc[:, n0:n1],
                                    qp8_sb[:, :, q0:q0 + 128],
                                    kl8_sb[:, :, k0 + n0:k0 + n1],
                                    start=False, stop=False,
                                    perf_mode=mybir.MatmulPerfMode.DoubleRow)
                            else:
                                nc.tensor.matmul(sc[:, n0:n1],
                                                 qT_sb[:, q0:q0 + 128],
                                                 kTl_sb[:, k0 + n0:k0 + n1],
                                                 start=False, stop=False)
                            if USE_FP8_P3:
                                nc.tensor.matmul(
                                    sc[:, n0:n1],
                                    r2l_sb[:, :, q0:q0 + 128],
                                    r2r_sb[:, :, k0 + n0:k0 + n1],
                                    start=False, stop=True,
                                    perf_mode=mybir.MatmulPerfMode.DoubleRow)
                            else:
                                nc.tensor.matmul(sc[:, n0:n1],
                                                 r2l_sb[:, q0:q0 + 128],
                                                 r2r_sb[:, k0 + n0:k0 + n1],
                                                 start=False, stop=True)
                        if c == nch - 1:
                            nc.vector.tensor_add(sc[:, kn - 128:kn],
                                                 sc[:, kn - 128:kn],
                                                 mask_sb[:])
                        PMAX = int(os.environ.get("K_PMAX", 0))
                        if split:
                            nc.vector.tensor_reduce(
                                mx[:, 5:6], sc[:, 512:kn],
                                axis=mybir.AxisListType.X, op=Alu.max)
                            nc.vector.tensor_reduce(
                                mx[:, c:c + 1], mx[:, 4:6],
                                axis=mybir.AxisListType.X, op=Alu.max)
                        elif PMAX and kn >= 256:
                            # pair-max: one PSUM pass reading two streams,
                            # then a 2x SBUF accum-max
                            half = kn // 2
                            ph = workp.tile([128, CHUNK], f32, tag="ph")
                            nc.vector.scalar_tensor_tensor(
                                ph[:, :half], sc[:, :half], 1.0,
                                sc[:, half:kn], Alu.mult, Alu.max)
                            nc.vector.tensor_scalar(
                                ph[:, CHUNK // 2:CHUNK // 2 + half],
                                ph[:, :half], 1.0, None,
                                Alu.mult, Alu.max,
                                accum_out=mx[:, c:c + 1])
                        else:
                            nc.vector.tensor_reduce(mx[:, c:c + 1],
                                                    sc[:, :kn],
                                                    axis=mybir.AxisListType.X,
                                                    op=Alu.max)
                        groups.append((sc, k0, 0, kn, False))
                        if nch > 1:
                            nmc = statp.tile([128, 1], f32, tag="nmc")
                            nc.vector.tensor_scalar(nmc[:], mx[:, c:c + 1],
                                                    nrho_sb[:, rb:rb + 1],
                                                    None, Alu.mult)
                            bias_ap = nmc[:]
                        else:
                            nc.vector.tensor_scalar(nm[:], mx[:, 0:1],
                                                    nrho_sb[:, rb:rb + 1],
                                                    LN16D,
                                                    Alu.mult, Alu.add)
                            bias_ap = nm[:]
                        nc.scalar.activation(e[:, k0:k0 + kn],
                                             sc[:, :kn],
                                             Act.Exp,
                                             bias=bias_ap,
                                             scale=rho_sb[:, rb:rb + 1],
                                             accum_out=zc[:, c:c + 1])
                    G = len(groups)
                    if G > 1:
                        m = statp.tile([128, 1], f32, tag="m")
                        nc.vector.tensor_reduce(m[:], mx[:, :G],
                                                axis=mybir.AxisListType.X,
                                                op=Alu.max)
                        nc.vector.tensor_scalar(nm[:], m[:],
                                                nrho_sb[:, rb:rb + 1],
                                                LN16D,
                                                Alu.mult, Alu.add)

                    r = statp.tile([128, 1], f32, tag="r")
                    t = workp.tile([128, S], bf16, tag="t")
                    eng = nc.gpsimd if MG_ENG == "g" else nc.vector
                    MGW = int(os.environ.get("K_MGW", 640))  # magic op width
                    if G > 1:
                        # s1_g = exp(rho*m_g + nm); winner = exactly 16*e^d
                        s1 = statp.tile([128, 6], f32, tag="s1")
                        nc.scalar.activation(s1[:, :G], mx[:, :G],
                                             Act.Exp,
                                             bias=nm[:],
                                             scale=rho_sb[:, rb:rb + 1])
                        w = statp.tile([128, 6], f32, tag="w")
                        nc.vector.tensor_tensor(w[:, :G], zc[:, :G],
                                                s1[:, :G], op=Alu.mult)
                        z = statp.tile([128, 1], f32, tag="z")
                        nc.vector.tensor_reduce(z[:], w[:, :G],
                                                axis=mybir.AxisListType.X,
                                                op=Alu.add)
                        nc.vector.reciprocal(r[:], z[:])
                        for g, (sc, k0, off, kn, diag) in enumerate(groups):
                            for m0 in range(0, kn, MGW):
                                m1 = min(kn, m0 + MGW)
                                eng.tensor_scalar(t[:, k0 + m0:k0 + m1],
                                                  e[:, k0 + m0:k0 + m1],
                                                  s1[:, g:g + 1], 127.5,
                                                  Alu.mult, Alu.add)
                    else:
                        nc.vector.reciprocal(r[:], zc[:, 0:1])
                        for m0 in range(0, NK, MGW):
                            m1 = min(NK, m0 + MGW)
                            eng.tensor_scalar(t[:, m0:m1], e[:, m0:m1],
                                              127.5, None, Alu.add)
                    return dict(t=t, r=r, groups=groups)

            def stage2(h, rb, ctx):
                    nonlocal gi
                    d = hdat[h]
                    v_sb = d["v"]
                    t = ctx["t"]
                    r = ctx["r"]
                    groups = ctx["groups"]
                    T = rb + 1
                    NK = T * 128

                    fT = workp.tile([128, NKT, 128], bf16, tag="fT")
                    if TRMODE == "dma":
                        # f = relu(t - 128) in SBUF (DVE 4x bf16), then the
                        # DMA crossbar transposes all T tiles in one shot
                        f = workp.tile([128, S], bf16, tag="f")
                        for g, (sc, k0, off, kn, diag) in enumerate(groups):
                            reng = nc.vector if RL_ENG == "v" else nc.gpsimd
                            reng.tensor_scalar(f[:, k0:k0 + kn],
                                               t[:, k0:k0 + kn],
                                               128.0, 0.0,
                                               Alu.subtract, Alu.max)
                        nc.sync.dma_start_transpose(fT[:, :T, :], f[:, :NK])
                    else:
                        for t0 in range(0, T, TG):
                            tn = min(TG, T - t0)
                            ptr = ps_t.tile([128, TG * 128], bf16, tag="tr")
                            for i in range(tn):
                                tt = t0 + i
                                nc.tensor.transpose(
                                    ptr[:, i * 128:(i + 1) * 128],
                                    t[:, tt * 128:(tt + 1) * 128],
                                    id_sb[:])
                            dst = fT[:, t0:t0 + tn, :]
                            if gi % EV_MOD == EV_ACT:
                                nc.scalar.activation(dst, ptr[:, :tn * 128],
                                                     Act.Relu,
                                                     bias=n128_sb[:],
                                                     scale=1.0)
                            else:
                                nc.vector.tensor_scalar(dst,
                                                        ptr[:, :tn * 128],
                                                        128.0, 0.0,
                                                        Alu.subtract, Alu.max)
                            gi += 1

                    pv = ps_v.tile([128, 128], f32, tag="pv")
                    for tt in range(T):
                        nc.tensor.matmul(pv[:],
                                         fT[:, tt, :],
                                         v_sb[:, tt, :],
                                         start=(tt == 0), stop=(tt == T - 1))

                    o = workp.tile([128, 128], f32, tag="o")
                    if os.environ.get("K_OSC", "a") == "a":
                        nc.scalar.mul(o[:], pv[:], r[:])
                    else:
                        nc.vector.tensor_scalar(o[:], pv[:], r[:], None,
                                                Alu.mult)
                    nc.sync.dma_start(out_d[h, rb], o[:])

            # software pipeline: emit stage1 of upcoming row-blocks before
            # stage2 of earlier ones so ready reduces/exps aren't queued
            # behind evictions that wait on the magic->transpose chain
            PIPE = int(os.environ.get("K_PIPE", 3))
            pend = []
            for (h, rb) in order:
                ctx = stage1(h, rb)
                pend.append((h, rb, ctx))
                if len(pend) >= PIPE:
                    h2, rb2, c2 = pend.pop(0)
                    stage2(h2, rb2, c2)
            for h2, rb2, c2 in pend:
                stage2(h2, rb2, c2)

    nc.compile()
    return nc


def _host_prep(query, key, value, qmin, qscale, kmin, kscale, vmin, vscale):
    """Builds per-head device inputs, stacked [H, ...]."""
    f32 = np.float32
    q = query[:, 0, :, :].astype(f32)     # [S, H, D]
    k = key[:, 0, :, :].astype(f32)
    v = value[:, 0, :, :].astype(f32)
    qs = qscale[:, 0, :].astype(f32)      # [S, H]
    qm = qmin[:, 0, :].astype(f32)
    ks = kscale[:, 0, :].astype(f32)
    km = kmin[:, 0, :].astype(f32)
    vs = vscale[:, 0, :, :].astype(f32)   # [G, H, D]
    vm = vmin[:, 0, :, :].astype(f32)

    rsd = f32(1.0 / math.sqrt(D))
    a = qs * rsd
    b = qm * rsd
    sq = q.sum(axis=2)
    sk = k.sum(axis=2)
    u = a * sq + b * f32(D)
    c = ks * sk

    # q side: a = rho * 2^e; q2e = q * 2^e exact in bf16.
    e_i = np.round(np.log2(a))
    two_e = np.exp2(e_i).astype(f32)
    rho = (a / two_e).astype(f32)
    q2e = q * two_e[:, :, None]                         # [S, H, D] exact
    qT = np.ascontiguousarray(q2e.transpose(1, 2, 0)).astype(BF16)  # [H,D,S]

    # scores PSUM is globally scaled by 2^GS; rho' = rho * 2^-GS
    rho_s = (rho * f32(2.0 ** -GS)).astype(f32)

    # k hi: bf16(ks*k) * 2^GS (exact exponent shift after rounding)
    kp = (k * ks[:, :, None]).astype(f32)
    kph = kp.astype(BF16).astype(f32)
    kTh = np.ascontiguousarray((kph * f32(2.0 ** GS)).transpose(1, 2, 0)
                               ).astype(BF16)           # [H, D, S]

    # k lo residual as fp8 pair path: q-pair (hi/lo nibbles of q2e * 2^QS8)
    q2e_s = (q2e * f32(2.0 ** QS8)).astype(f32)
    q1 = q2e_s.astype(FP8E4).astype(f32)
    q2 = (q2e_s - q1).astype(FP8E4)
    qp8 = np.stack([q1.astype(FP8E4), q2], axis=2)      # [S, H, 2, D]
    qp8 = np.ascontiguousarray(qp8.transpose(1, 3, 2, 0))  # [H, D, 2, S]

    kpl = ((kp - kph) * f32(2.0 ** KS8)).astype(FP8E4)  # [S, H, D]
    kl8 = np.stack([kpl, kpl], axis=2)                  # [S, H, 2, D]
    kl8 = np.ascontiguousarray(kl8.transpose(1, 3, 2, 0))  # [H, D, 2, S]

    rho_r = np.ascontiguousarray(
        rho_s.T.reshape(H, NRB, 128).transpose(0, 2, 1)).astype(f32)
    nrho_r = np.ascontiguousarray(-rho_r)

    def hilo(x):
        xh = x.astype(BF16).astype(f32)
        xl = (x - xh).astype(BF16).astype(f32)
        return xh, xl

    # rank-6 correction, scaled 2^GS split as 2^9 per side
    s9 = f32(2.0 ** 9)
    up = (u / rho).astype(f32)
    bp = (b / rho).astype(f32)
    uh, ul = hilo(up * s9)
    bh, bl = hilo(bp * s9)
    kmh, kml = hilo(km * s9)
    ch, cl = hilo(c * s9)
    r2l = np.stack([uh, uh, ul, bh, bh, bl], axis=0)     # [6, S, H]
    r2r = np.stack([kmh, kml, kmh, ch, cl, ch], axis=0)
    r2l = np.ascontiguousarray(r2l.transpose(2, 0, 1)).astype(BF16)  # [H,6,S]
    r2r = np.ascontiguousarray(r2r.transpose(2, 0, 1)).astype(BF16)

    # corr as fp8e5 DoubleRow level pairs: corr*2^GS = sum over PAIRS of
    # (upL_i*2^a)(kmL_j*2^(GS-a)) + (bpL_i*2^a)(cL_j*2^(GS-a))
    def e5_levels(x):
        parts = []
        rr = x.astype(f32).copy()
        for _ in range(NLV):
            mmx = max(float(np.abs(rr).max()), 1e-30)
            sh = f32(2.0 ** np.floor(np.log2(28672.0 / mmx)))
            p = (rr * sh).astype(FP8E5).astype(f32) / sh
            parts.append(p)
            rr = rr - p
        return parts

    upL = e5_levels(up)
    bpL = e5_levels(bp)
    kmL = e5_levels(km)
    cL = e5_levels(c)

    def pair_rows(lv_list, rv_list):
        lrows = np.zeros((NPAIR, S, H), dtype=FP8E5)
        rrows = np.zeros((NPAIR, S, H), dtype=FP8E5)
        for p, (i, j) in enumerate(PAIRS):
            lv, rv = lv_list[i], rv_list[j]
            ml = max(float(np.abs(lv).max()), 1e-30)
            mr = max(float(np.abs(rv).max()), 1e-30)
            al = np.round((GS + np.log2(mr) - np.log2(ml)) / 2.0)
            al = min(al, np.floor(np.log2(57344.0 / ml)))
            al = max(al, GS - np.floor(np.log2(57344.0 / mr)))
            lrows[p] = (lv * f32(2.0 ** al)).astype(FP8E5)
            rrows[p] = (rv * f32(2.0 ** (GS - al))).astype(FP8E5)
        return lrows, rrows

    la, ra = pair_rows(upL, kmL)
    lb, rb_ = pair_rows(bpL, cL)
    r3l = np.stack([la, lb], axis=1)                     # [NPAIR, 2, S, H]
    r3r = np.stack([ra, rb_], axis=1)
    r3l = np.ascontiguousarray(r3l.transpose(3, 0, 1, 2))  # [H, NPAIR, 2, S]
    r3r = np.ascontiguousarray(r3r.transpose(3, 0, 1, 2))

    vs_full = np.repeat(vs, VG, axis=0)
    vm_full = np.repeat(vm, VG, axis=0)
    vd = v * vs_full + vm_full            # f32 [S, H, D]
    vdt = vd.transpose(1, 0, 2).reshape(H, NKT, 128, D)
    vdt = np.ascontiguousarray(vdt.transpose(0, 2, 1, 3)).astype(BF16)

    mask = np.triu(np.full((128, 128), -1e30, dtype=f32), k=1)
    ident = np.eye(128, dtype=np.float32).astype(BF16)

    return dict(qT=qT, kTh=kTh, qp8=qp8, kl8=kl8, rho=rho_r, nrho=nrho_r,
                r2l=r2l, r2r=r2r, r3l=r3l, r3r=r3r, vv=vdt, mask=mask,
                neg128=np.full((128, 1), -128.0, dtype=f32),
                ident=ident, vd_f32=vd)


def _host_last_row(query, key, qmin, qscale, kmin, kscale, vd_f32):
    """Exact reference math (numpy f32) for the single non-causal row."""
    f32 = np.float32
    i = S - 1
    out = np.zeros((H, D), dtype=f32)
    for h in range(H):
        qd = query[i, 0, h, :].astype(f32) * f32(qscale[i, 0, h]) + f32(qmin[i, 0, h])
        kd = key[:, 0, h, :].astype(f32) * kscale[:, 0, h].astype(f32)[:, None] \
            + kmin[:, 0, h].astype(f32)[:, None]
        s = (kd @ qd).astype(f32) * f32(1.0 / math.sqrt(D))
        e = np.exp(s - s.max(), dtype=f32)
        p = (e / e.sum(dtype=f32)).astype(f32)
        pmax, pmin_ = p.max(), p.min()
        pscale = (pmax - pmin_) / f32(P_LEVELS)
        safe = pscale if pscale > 0 else f32(1.0)
        pq = np.floor((p - pmin_) / safe).astype(f32)
        pd = pq * pscale + pmin_
        out[h] = pd @ vd_f32[:, h, :]
    return out


def _reference_numpy(query, key, value, qmin, qscale, kmin, kscale,
                     vmin, vscale, causal):
    f32 = np.float32
    q = query[:, 0, :, :].astype(f32)
    k = key[:, 0, :, :].astype(f32)
    v = value[:, 0, :, :].astype(f32)
    out = np.zeros((S, B, H * D), dtype=f32)
    vs_full = np.repeat(vscale[:, 0, :, :].astype(f32), VG, axis=0)
    vm_full = np.repeat(vmin[:, 0, :, :].astype(f32), VG, axis=0)
    for h in range(H):
        qd = q[:, h, :] * qscale[:, 0, h].astype(f32)[:, None] + qmin[:, 0, h].astype(f32)[:, None]
        kd = k[:, h, :] * kscale[:, 0, h].astype(f32)[:, None] + kmin[:, 0, h].astype(f32)[:, None]
        s = (qd @ kd.T) * f32(1.0 / math.sqrt(D))
        if causal:
            s = np.where(np.tril(np.ones((S, S), dtype=bool)), s, f32(-1e30))
        e = np.exp(s - s.max(axis=1, keepdims=True), dtype=f32)
        p = e / e.sum(axis=1, keepdims=True, dtype=f32)
        pmax = p.max(axis=1, keepdims=True)
        pmin_ = p.min(axis=1, keepdims=True)
        pscale = (pmax - pmin_) / f32(P_LEVELS)
        safe = np.where(pscale > 0, pscale, f32(1.0))
        pd = np.floor((p - pmin_) / safe) * pscale + pmin_
        vd = v[:, h, :] * vs_full[:, h, :] + vm_full[:, h, :]
        out[:, 0, h * D:(h + 1) * D] = pd.astype(f32) @ vd
    return out


def kernel(query, key, value, qmin, qscale, kmin, kscale, vmin, vscale,
           causal):
    global _COMPILED
    causal_i = int(np.asarray(causal))
    if causal_i != 1:
        return _reference_numpy(query, key, value, qmin, qscale, kmin,
                                kscale, vmin, vscale, causal_i)

    prep = _host_prep(query, key, value, qmin, qscale, kmin, kscale,
                      vmin, vscale)

    if _COMPILED is None:
        _COMPILED = _build_graph()
    nc = _COMPILED

    in_maps = []
    for core in range(N_CORES):
        hs = slice(core * HPC, (core + 1) * HPC)
        in_maps.append({
            "qT": np.ascontiguousarray(prep["qT"][hs]),
            "kTh": np.ascontiguousarray(prep["kTh"][hs]),
            "qp8": np.ascontiguousarray(prep["qp8"][hs]),
            "kl8": np.ascontiguousarray(prep["kl8"][hs]),
            "rho": np.ascontiguousarray(prep["rho"][hs]),
            "nrho": np.ascontiguousarray(prep["nrho"][hs]),
            "r2l": np.ascontiguousarray(prep["r2l"][hs]),
            "r2r": np.ascontiguousarray(prep["r2r"][hs]),
            "r3l": np.ascontiguousarray(prep["r3l"][hs]),
            "r3r": np.ascontiguousarray(prep["r3r"][hs]),
            "vv": np.ascontiguousarray(prep["vv"][hs]),
            "mask": prep["mask"],
            "neg128": prep["neg128"],
            "ident": prep["ident"],
        })

    from concourse.bass_utils import run_bass_kernel_spmd
    trace = bool(int(os.environ.get("KERNEL_TRACE", "0")))
    res = run_bass_kernel_spmd(nc, in_maps, core_ids=list(range(N_CORES)),
                               trace=trace)
    if res.exec_time_ns is not None:
        kernel.last_exec_ns = res.exec_time_ns
        print(f"HW exec time: {res.exec_time_ns} ns")

    out = np.zeros((S, B, H * D), dtype=np.float32)
    for core in range(N_CORES):
        o = np.asarray(res.results[core]["out"], dtype=np.float32)
        for j in range(HPC):
            h = core * HPC + j
            out[:, 0, h * D:(h + 1) * D] = o[j].reshape(S, D)

    last = _host_last_row(query, key, qmin, qscale, kmin, kscale,
                          prep["vd_f32"])
    for h in range(H):
        out[S - 1, 0, h * D:(h + 1) * D] = last[h]
    return out


kernel.last_exec_ns = None
